# revision 1
# baseline (speedup 1.0000x reference)
"""Trainium2 Bass kernel for nn_CutLayer (histogram_binning).

Strategy (data-parallel over events, 8 cores):
  L1: per-core min/max of the feature column (device reduce).
  L2: per-core exact per-edge cumulative counts, split across two engines:
      - Vector (DVE): scalar_tensor_tensor (x <= e) * w with w = 1 + 4096*y,
        per-partition accumulated over 3906-element halves, packing
        count and signal-count into one exact fp32 integer.
      - Scalar (ACT): sign(x - e) with per-partition accumulation on both
        the full stream and a signal-masked stream; counts recovered as
        (N + ties - sum_sign) / 2 (exact +-1 sums).
  host: combine counts, repair lt/le tie counts from a tiny candidate set,
      replicate the reference's tiny E^2 pair search bit-exactly with
      eager CPU jax, producing (lower, upper, case).
  L3: per-core case-specialized predicate (4 lazily-built programs; only
      the dispatched case compiles): cases 0/1 are a single 2x-rate
      tensor_scalar compare; cases 2/3 are one compare plus one fused
      scalar_tensor_tensor combine. All compares exact.

Events per core: 1_000_000; the device handles 128*7812 = 999_936 of them
(SBUF tile [128, 7812]); the 64-per-core remainder is handled exactly on
the host (512 events total).
"""

from contextlib import ExitStack

import numpy as np

import concourse.bass as bass
import concourse.mybir as mybir
from concourse.bass_utils import run_bass_kernel_spmd

N = 8_000_000
N_CORES = 8
CORE_N = N // N_CORES            # 1_000_000
P = 128
F = 7812                         # free-dim columns per partition
H = F // 2                       # packed-accum half (counts < 4096)
DEV_N = P * F                    # 999_936 device events per core
N_DEV_TOT = DEV_N * N_CORES      # 7_999_488
N_BINS = 50
E = N_BINS + 1                   # 51 edges
EPS = 1e-7
KD = 32                          # edges handled by the vector engine
KA = E - KD                      # edges handled by the scalar engine
PACK = 4096.0                    # signal-count multiplier (exact < 2^24)

FP32 = mybir.dt.float32
BF16 = mybir.dt.bfloat16
I32 = mybir.dt.int32
AX = mybir.AxisListType
OP = mybir.AluOpType
ACT = mybir.ActivationFunctionType

CORE_IDS = list(range(N_CORES))


# --------------------------------------------------------------------------
# Bass programs (built once per process)
# --------------------------------------------------------------------------

def _build_minmax():
    nc = bass.Bass()
    x = nc.declare_dram_parameter("x", [DEV_N], FP32, isOutput=False)
    mn = nc.declare_dram_parameter("mn", [P], FP32, isOutput=True)
    mx = nc.declare_dram_parameter("mx", [P], FP32, isOutput=True)
    with (
        nc.sbuf_tensor([P, F], FP32) as xt,
        nc.sbuf_tensor([P, 2], FP32) as acc,
        nc.semaphore() as dsem,
        nc.semaphore() as csem,
        nc.Block() as block,
    ):
        @block.sync
        def _(sync):
            sync.dma_start(xt[:], x[:].rearrange("(p f) -> p f", p=P)).then_inc(
                dsem, 16
            )
            sync.wait_ge(csem, 2)
            sync.dma_start(mn[:], acc[:, 0:1]).then_inc(dsem, 16)
            sync.dma_start(mx[:], acc[:, 1:2]).then_inc(dsem, 16)
            sync.wait_ge(dsem, 48)

        @block.vector
        def _(vector):
            vector.wait_ge(dsem, 16)
            vector.tensor_reduce(acc[:, 0:1], xt[:], axis=AX.X, op=OP.min).then_inc(
                csem, 1
            )
            vector.tensor_reduce(acc[:, 1:2], xt[:], axis=AX.X, op=OP.max).then_inc(
                csem, 1
            )
    return nc


def _build_counts():
    nc = bass.Bass()
    x = nc.declare_dram_parameter("x", [DEV_N], FP32, isOutput=False)
    w = nc.declare_dram_parameter("w", [DEV_N], FP32, isOutput=False)
    xs = nc.declare_dram_parameter("xs", [DEV_N], FP32, isOutput=False)
    ed = nc.declare_dram_parameter("edges", [P, 2 * E], FP32, isOutput=False)
    opk = nc.declare_dram_parameter("acc_pk", [P, 2 * KD], FP32, isOutput=True)
    osa = nc.declare_dram_parameter("acc_sa", [P, KA], FP32, isOutput=True)
    oss = nc.declare_dram_parameter("acc_ss", [P, KA], FP32, isOutput=True)
    with ExitStack() as es:
        ec = es.enter_context
        xt = ec(nc.sbuf_tensor([P, F], FP32))
        wt = ec(nc.sbuf_tensor([P, F], FP32))
        xst = ec(nc.sbuf_tensor([P, F], FP32))
        scr = ec(nc.sbuf_tensor([P, F], FP32))
        asca = ec(nc.sbuf_tensor([P, F], BF16))
        ascb = ec(nc.sbuf_tensor([P, F], BF16))
        edt = ec(nc.sbuf_tensor([P, 2 * E], FP32))
        apk = ec(nc.sbuf_tensor([P, 2 * KD], FP32))
        asa = ec(nc.sbuf_tensor([P, KA], FP32))
        ass = ec(nc.sbuf_tensor([P, KA], FP32))
        names = ["dse", "dsx", "dsx1", "dsw", "dsw1", "dss", "dso",
                 "csem", "v0", "v1", "t0", "t1"]
        dse, dsx, dsx1, dsw, dsw1, dss, dso, csem, v0, v1, t0, t1 = (
            ec(nc.semaphore(n)) for n in names
        )
        block = ec(nc.Block())
        @block.sync
        def _(sync):
            xv = x[:].rearrange("(p f) -> p f", p=P)
            wv = w[:].rearrange("(p f) -> p f", p=P)
            sync.dma_start(edt[:], ed[:]).then_inc(dse, 16)
            # interleave x/w halves so the vector engine can start on the
            # first data half as soon as possible
            sync.dma_start(xt[:, 0:H], xv[:, 0:H]).then_inc(dsx, 16)
            sync.dma_start(wt[:, 0:H], wv[:, 0:H]).then_inc(dsw, 16)
            sync.dma_start(xt[:, H:F], xv[:, H:F]).then_inc(dsx1, 16)
            sync.dma_start(wt[:, H:F], wv[:, H:F]).then_inc(dsw1, 16)
            sync.dma_start(xst[:], xs[:].rearrange("(p f) -> p f", p=P)).then_inc(
                dss, 16
            )
            # ACT typically retires first: ship its accumulators while the
            # vector engine finishes, then the packed accumulators.
            sync.wait_ge(t0, 2 * ((KA + 1) // 2))
            sync.wait_ge(t1, 2 * (KA // 2))
            sync.dma_start(osa[:], asa[:]).then_inc(dso, 16)
            sync.dma_start(oss[:], ass[:]).then_inc(dso, 16)
            sync.wait_ge(v0, KD)
            sync.wait_ge(v1, KD)
            sync.dma_start(opk[:], apk[:]).then_inc(dso, 16)
            sync.wait_ge(dso, 48)

        @block.vector
        def _(vector):
            # phase 1: all edges on data half 0 (needs edges + x0 + w0),
            # phase 2: all edges on data half 1 — accumulator slots are
            # per (edge, half) so order is free. Scratch regions ping-pong
            # by instruction parity with retirement semaphores.
            vector.wait_ge(dse, 16)
            vector.wait_ge(dsx, 16)
            vector.wait_ge(dsw, 16)
            nh = [0, 0]  # completed instruction count per scratch region
            ninstr = 0
            for dh in range(2):
                hs = dh * H
                if dh == 1:
                    vector.wait_ge(dsx1, 16)
                    vector.wait_ge(dsw1, 16)
                for i in range(KD):
                    e = edt[:, i : i + 1]
                    rr = ninstr % 2
                    if nh[rr] >= 1:
                        vector.wait_ge([v0, v1][rr], nh[rr])
                    vector.scalar_tensor_tensor(
                        scr[:, rr * H : rr * H + H],
                        xt[:, hs : hs + H],
                        e,
                        wt[:, hs : hs + H],
                        op0=OP.is_le,
                        op1=OP.mult,
                        accum_out=apk[:, 2 * i + dh : 2 * i + dh + 1],
                    ).then_inc([v0, v1][rr], 1)
                    nh[rr] += 1
                    ninstr += 1

        @block.scalar
        def _(scalar):
            # x-stream first (needs edges + x only)
            scalar.wait_ge(dse, 16)
            scalar.wait_ge(dsx, 16)
            scalar.wait_ge(dsx1, 16)
            na = [0, 0]
            scrs = [asca, ascb]
            sems = [t0, t1]
            for i in range(KA):
                ne = edt[:, E + KD + i : E + KD + i + 1]  # negated edge
                hh = i % 2
                if na[hh] >= 1:
                    scalar.wait_ge(sems[hh], na[hh])
                scalar.activation(
                    scrs[hh][:], xt[:], ACT.Sign, bias=ne, scale=1.0,
                    accum_out=asa[:, i : i + 1],
                ).then_inc(sems[hh], 1)
                na[hh] += 1
            # signal stream (needs xs)
            scalar.wait_ge(dss, 16)
            for i in range(KA):
                ne = edt[:, E + KD + i : E + KD + i + 1]
                hh = i % 2
                if na[hh] >= 1:
                    scalar.wait_ge(sems[hh], na[hh])
                scalar.activation(
                    scrs[hh][:], xst[:], ACT.Sign, bias=ne, scale=1.0,
                    accum_out=ass[:, i : i + 1],
                ).then_inc(sems[hh], 1)
                na[hh] += 1
    return nc


def _build_pred(case: int):
    """Case-specialized predicate:
    0: x <= lo                    (1 pass)
    1: x >= lo                    (1 pass)
    2: (x >= lo) & (x <= up)      (2 passes)
    3: (x <= lo) | (x >= up)      (2 passes, disjoint -> add)
    """
    nc = bass.Bass()
    x = nc.declare_dram_parameter("x", [DEV_N], FP32, isOutput=False)
    pr = nc.declare_dram_parameter("prm", [P, 8], FP32, isOutput=False)
    out = nc.declare_dram_parameter("pred", [DEV_N], I32, isOutput=True)
    with (
        nc.sbuf_tensor([P, F], FP32) as xt,
        nc.sbuf_tensor([P, F], FP32) as t,
        nc.sbuf_tensor([P, F], I32) as pi,
        nc.sbuf_tensor([P, 8], FP32) as prm,
        nc.semaphore() as dsem,
        nc.semaphore() as csem,
        nc.semaphore() as tsem,
        nc.Block() as block,
    ):
        @block.sync
        def _(sync):
            sync.dma_start(prm[:], pr[:]).then_inc(dsem, 16)
            sync.dma_start(xt[:], x[:].rearrange("(p f) -> p f", p=P)).then_inc(
                dsem, 16
            )
            sync.wait_ge(csem, 1)
            sync.dma_start(
                out[:].rearrange("(p f) -> p f", p=P), pi[:]
            ).then_inc(dsem, 16)
            sync.wait_ge(dsem, 48)

        @block.vector
        def _(vector):
            vector.wait_ge(dsem, 32)
            lo = prm[:, 0:1]
            up = prm[:, 1:2]
            if case == 0:
                vector.tensor_scalar(pi[:], xt[:], lo, None, OP.is_le).then_inc(
                    csem, 1
                )
            elif case == 1:
                vector.tensor_scalar(pi[:], xt[:], lo, None, OP.is_ge).then_inc(
                    csem, 1
                )
            elif case == 2:
                vector.tensor_scalar(t[:], xt[:], up, None, OP.is_le).then_inc(
                    tsem, 1
                )
                vector.wait_ge(tsem, 1)
                vector.scalar_tensor_tensor(
                    pi[:], xt[:], lo, t[:], op0=OP.is_ge, op1=OP.mult
                ).then_inc(csem, 1)
            else:
                vector.tensor_scalar(t[:], xt[:], up, None, OP.is_ge).then_inc(
                    tsem, 1
                )
                vector.wait_ge(tsem, 1)
                vector.scalar_tensor_tensor(
                    pi[:], xt[:], lo, t[:], op0=OP.is_le, op1=OP.add
                ).then_inc(csem, 1)
    return nc


_PROGRAMS: dict = {}


def _prog(name):
    if name not in _PROGRAMS:
        if name.startswith("pred"):
            _PROGRAMS[name] = _build_pred(int(name[4:]))
        else:
            _PROGRAMS[name] = {
                "minmax": _build_minmax,
                "counts": _build_counts,
            }[name]()
    return _PROGRAMS[name]


# --------------------------------------------------------------------------
# Host orchestration
# --------------------------------------------------------------------------

LAST_EXEC_NS: list = []


_CACHE_SET = False


def _enable_jit_cache():
    # Persist compiled executables (which embed the NEFF) across processes;
    # harmless no-op if the backend doesn't support serialization.
    global _CACHE_SET
    if _CACHE_SET:
        return
    _CACHE_SET = True
    try:
        import jax

        jax.config.update("jax_compilation_cache_dir", "/tmp/jax_bass_cache")
        jax.config.update("jax_persistent_cache_min_compile_time_secs", 1.0)
        jax.config.update("jax_persistent_cache_min_entry_size_bytes", 0)
    except Exception:
        pass


def _run(name, in_maps):
    import os

    _enable_jit_cache()
    trace = bool(int(os.environ.get("BASS_KERNEL_PROFILE", "0")))
    r = run_bass_kernel_spmd(_prog(name), in_maps, CORE_IDS, trace=trace)
    if trace:
        LAST_EXEC_NS.append((name, r.exec_time_ns, r.mean_exec_time_ns))
    return r.results


def _dev_shard(arr, c):
    return arr[c * CORE_N : c * CORE_N + DEV_N]


def _tail_shard(arr, c):
    return arr[c * CORE_N + DEV_N : (c + 1) * CORE_N]


def kernel(inputs: np.ndarray, targets: np.ndarray) -> np.ndarray:
    x_full = np.ascontiguousarray(inputs[:, 0]).astype(np.float32, copy=False)
    y_full = np.asarray(targets)

    tails_x = [_tail_shard(x_full, c) for c in CORE_IDS]
    tails_y = [_tail_shard(y_full, c) for c in CORE_IDS]
    tail_x = np.concatenate(tails_x)
    tail_y = np.concatenate(tails_y)

    # ---- L1: global min/max -------------------------------------------------
    LAST_EXEC_NS.clear()
    res1 = _run("minmax", [{"x": _dev_shard(x_full, c)} for c in CORE_IDS])
    gmin = np.float32(min(min(r["mn"].min() for r in res1), tail_x.min()))
    gmax = np.float32(max(max(r["mx"].max() for r in res1), tail_x.max()))

    # ---- edges: replicate jnp.linspace bit-exactly (eager CPU jax) ----------
    import jax
    import jax.numpy as jnp

    cpu = jax.devices("cpu")[0]
    with jax.default_device(cpu):
        edges = np.asarray(jnp.linspace(jnp.float32(gmin), jnp.float32(gmax), E))

    # ---- L2: per-edge counts ------------------------------------------------
    sig_mask = y_full == 1
    # Finite sentinel above every possible edge (sim paths reject inf inputs).
    sent = np.float32(np.finfo(np.float32).max)
    x_sig = np.where(sig_mask, x_full, sent).astype(np.float32)
    w_full = (1.0 + PACK * sig_mask).astype(np.float32)
    ed_in = np.concatenate([edges, -edges]).astype(np.float32)
    edges_rep = np.ascontiguousarray(np.broadcast_to(ed_in, (P, 2 * E)))

    res2 = _run(
        "counts",
        [
            {
                "x": _dev_shard(x_full, c),
                "w": _dev_shard(w_full, c),
                "xs": _dev_shard(x_sig, c),
                "edges": edges_rep,
            }
            for c in CORE_IDS
        ],
    )

    # ---- exact tie counts (x == edge) from a tiny candidate set -------------
    h = (np.float32(gmax) - np.float32(gmin)) / np.float32(N_BINS)
    inv_h = np.float32(1.0) / h if h != 0 else np.float32(0.0)
    u = (x_full - gmin) * inv_h
    r_near = np.rint(u)
    cand = np.abs(u - r_near) < np.float32(0.01)
    idx = np.flatnonzero(cand)
    T_all = np.zeros(E, np.float64)
    Tsig_all = np.zeros(E, np.float64)
    T_dev = np.zeros(E, np.float64)
    Tsig_dev = np.zeros(E, np.float64)
    if idx.size:
        kn = np.clip(r_near[idx].astype(np.int64), 0, E - 1)
        is_tie = x_full[idx] == edges[kn]
        tidx = idx[is_tie]
        tie_k = kn[is_tie]
        tie_sig = sig_mask[tidx]
        tie_dev = (tidx % CORE_N) < DEV_N
        np.add.at(T_all, tie_k, 1.0)
        np.add.at(Tsig_all, tie_k[tie_sig], 1.0)
        np.add.at(T_dev, tie_k[tie_dev], 1.0)
        np.add.at(Tsig_dev, tie_k[tie_dev & tie_sig], 1.0)

    # ---- decode device counts ----------------------------------------------
    cnt_le = np.zeros(E, np.float64)
    sig_le = np.zeros(E, np.float64)
    cnt_pk = np.zeros(2 * KD, np.int64)
    sig_pk = np.zeros(2 * KD, np.int64)
    sa = np.zeros(KA, np.float64)
    ss = np.zeros(KA, np.float64)
    for r in res2:
        # decode per accumulator slot (each packs cnt<4096 with 4096*sig)
        a = r["acc_pk"].astype(np.int64)
        s_part = a // int(PACK)
        c_part = a - int(PACK) * s_part
        cnt_pk += c_part.sum(axis=0)
        sig_pk += s_part.sum(axis=0)
        sa += r["acc_sa"].astype(np.float64).sum(axis=0)
        ss += r["acc_ss"].astype(np.float64).sum(axis=0)
    cnt_le[:KD] = cnt_pk.reshape(KD, 2).sum(axis=1)
    sig_le[:KD] = sig_pk.reshape(KD, 2).sum(axis=1)
    cnt_le[KD:] = (N_DEV_TOT + T_dev[KD:] - sa) / 2.0
    sig_le[KD:] = (N_DEV_TOT + Tsig_dev[KD:] - ss) / 2.0

    # tail events, exact
    cnt_le += (tail_x[:, None] <= edges[None, :]).sum(axis=0)
    sig_le += (tail_x[tail_y == 1][:, None] <= edges[None, :]).sum(axis=0)

    cnt_lt = cnt_le - T_all
    sig_lt = sig_le - Tsig_all

    ns_le = sig_le.astype(np.float32)
    ns_lt = sig_lt.astype(np.float32)
    nb_le = (cnt_le - sig_le).astype(np.float32)
    nb_lt = (cnt_lt - sig_lt).astype(np.float32)

    # ---- replicate the reference's tiny pair search (eager CPU jax) ---------
    with jax.default_device(cpu):
        ns_le_j = jnp.asarray(ns_le)
        ns_lt_j = jnp.asarray(ns_lt)
        nb_le_j = jnp.asarray(nb_le)
        nb_lt_j = jnp.asarray(nb_lt)
        n_f = jnp.float32(N)
        Ns = ns_le_j[-1]
        Nb = n_f - Ns

        hist0 = nb_le_j[1:] - nb_lt_j[:-1]
        hist1 = ns_le_j[1:] - ns_lt_j[:-1]

        gt0 = hist0 > hist1
        cand0 = jnp.logical_xor(gt0[:-1], gt0[1:]) & (hist0[:-1] > 0)
        gt1 = hist1 > hist0
        cand1 = jnp.logical_xor(gt1[:-1], gt1[1:]) & (hist1[:-1] > 0)
        mask = jnp.zeros((E,), bool).at[1:N_BINS].set(cand0 | cand1)
        cnt = jnp.sum(mask)
        mask = mask.at[-1].set(mask[-1] | (cnt == 1))

        a_c = -jnp.log1p(jnp.float32(-EPS))
        b_c = -jnp.log(jnp.float32(EPS))

        def bce(correct):
            return ((n_f - correct) * b_c + correct * a_c) / n_f

        c0 = ns_le_j + (Nb - nb_le_j)
        c1 = (Ns - ns_lt_j) + nb_lt_j
        c2 = (ns_le_j[None, :] - ns_lt_j[:, None]) + Nb - (
            nb_le_j[None, :] - nb_lt_j[:, None]
        )
        c3 = ns_le_j[:, None] + (Ns - ns_lt_j[None, :]) + (
            nb_le_j[None, :] - nb_lt_j[:, None]
        )

        L = jnp.stack(
            [
                jnp.broadcast_to(bce(c0)[:, None], (E, E)),
                jnp.broadcast_to(bce(c1)[:, None], (E, E)),
                bce(c2),
                bce(c3),
            ]
        )
        per_pair_min = jnp.min(L, axis=0)
        per_pair_case = jnp.argmin(L, axis=0)

        idxs = jnp.arange(E)
        valid = mask[:, None] & mask[None, :] & (idxs[:, None] < idxs[None, :])
        flat = jnp.argmin(jnp.where(valid, per_pair_min, jnp.inf))
        i = int(flat) // E
        j = int(flat) % E
        lower = np.float32(edges[i])
        upper = np.float32(edges[j])
        case = int(per_pair_case[i, j])

    # ---- L3: predicate (case-specialized program; exact compares) --------
    prm = np.zeros((P, 8), np.float32)
    prm[:, 0] = lower
    prm[:, 1] = upper

    res3 = _run(
        f"pred{case}", [{"x": _dev_shard(x_full, c), "prm": prm} for c in CORE_IDS]
    )

    out = np.empty(N, np.int32)
    for c in CORE_IDS:
        out[c * CORE_N : c * CORE_N + DEV_N] = res3[c]["pred"]
        tx = tails_x[c]
        if case == 0:
            tp = tx <= lower
        elif case == 1:
            tp = tx >= lower
        elif case == 2:
            tp = (tx >= lower) & (tx <= upper)
        else:
            tp = (tx <= lower) | (tx >= upper)
        out[c * CORE_N + DEV_N : (c + 1) * CORE_N] = tp.astype(np.int32)
    return out



# revision 4
# speedup vs baseline: 1.0009x; 1.0009x over previous
"""Trainium2 Bass kernel for nn_CutLayer (histogram_binning) — v2.

Strategy (data-parallel over events, 8 cores):
  Host prep: extract feature column, quantize to fp16 (exact-repairable),
    class-compact globally (all signal events first, then background) so
    signal counts fall out of the per-partition accumulator structure.
  L1 minmax: per-partition min/max of the fp16 stream (DVE fold + reduce);
    host reconstructs the exact fp32 min/max from the tiny fp16-min bucket.
  L2 counts: per-edge cumulative counts via a single fp16 pass per edge:
    - DVE: tensor_scalar is_le with fp32 accumulator (4x perf mode).
    - ACT: Sign(x - e) with fp32 accumulator (1x, takes the leftover edges).
    Edges 0 and 50 equal the min/max and are host-derived; only 49 edges
    run on device. Counts are exact in fp16-space; the host repairs them
    to fp32-space using a small candidate band around each edge (all
    elements whose fp16 rounding could flip any compare), then runs the
    reference's tiny E^2 pair search bit-exactly with eager CPU jax.
  L3 pred: case-specialized predicate in fp16, chunked so the output DMA
    overlaps compute; host patches the band around the chosen thresholds
    and inverse-permutes back to event order.

Events: 8_000_000 total; device handles 8 * 128 * 7812 = 7_999_488 in
compacted order; the 512-element tail (all background) is host-exact.
"""

import os
from contextlib import ExitStack

import numpy as np

import concourse.bass as bass
import concourse.mybir as mybir
from concourse.bass_utils import run_bass_kernel_spmd

N = 8_000_000
N_CORES = 8
P = 128
F = 7812                         # free-dim columns per partition
H = F // 2
Q = F // 4
DEV_N = P * F                    # 999_936 device events per core
N_DEV_TOT = DEV_N * N_CORES      # 7_999_488
N_BINS = 50
E = N_BINS + 1                   # 51 edges
EPS = 1e-7
DEVE = list(range(1, E - 1))     # 49 device edges (0 and 50 host-derived)
NV = 37                          # edges on the vector engine (4x fp16)
DVE_EDGES = DEVE[:NV]
ACT_EDGES = DEVE[NV:]
NA = len(ACT_EDGES)

FP32 = mybir.dt.float32
FP16 = mybir.dt.float16
BF16 = mybir.dt.bfloat16
AX = mybir.AxisListType
OP = mybir.AluOpType
ACT = mybir.ActivationFunctionType

CORE_IDS = list(range(N_CORES))

# fp16 min normal; |x| below this is routed through the host (sentinel 0.0
# on device) so fp16-subnormal flush behaviour can never matter.
F16_TINY = 6.2e-5


# --------------------------------------------------------------------------
# Bass programs
# --------------------------------------------------------------------------

def _build_minmax():
    nc = bass.Bass()
    x = nc.declare_dram_parameter("x", [DEV_N], FP16, isOutput=False)
    mm = nc.declare_dram_parameter("mm", [P, 2], FP16, isOutput=True)
    with (
        nc.sbuf_tensor([P, F], FP16) as xt,
        nc.sbuf_tensor([P, H], FP16) as s0,
        nc.sbuf_tensor([P, H], FP16) as s1,
        nc.sbuf_tensor([P, 2], FP16) as acc,
        nc.semaphore() as dsem,
        nc.semaphore() as csem,
        nc.Block() as block,
    ):
        @block.sync
        def _(sync):
            sync.dma_start(xt[:], x[:].rearrange("(p f) -> p f", p=P)).then_inc(
                dsem, 16
            )
            sync.wait_ge(csem, 4)
            sync.dma_start(mm[:], acc[:]).then_inc(dsem, 16)
            sync.wait_ge(dsem, 32)

        @block.vector
        def _(vector):
            vector.wait_ge(dsem, 16)
            vector.tensor_tensor(s0[:], xt[:, 0:H], xt[:, H:F], op=OP.min).then_inc(
                csem, 1
            )
            vector.tensor_reduce(acc[:, 0:1], s0[:], axis=AX.X, op=OP.min).then_inc(
                csem, 1
            )
            vector.tensor_tensor(s1[:], xt[:, 0:H], xt[:, H:F], op=OP.max).then_inc(
                csem, 1
            )
            vector.tensor_reduce(acc[:, 1:2], s1[:], axis=AX.X, op=OP.max).then_inc(
                csem, 1
            )
    return nc


def _build_counts():
    nc = bass.Bass()
    x = nc.declare_dram_parameter("x", [DEV_N], FP16, isOutput=False)
    ed = nc.declare_dram_parameter("edges", [P, 2 * E], FP32, isOutput=False)
    oad = nc.declare_dram_parameter("acc_dve", [P, 2 * NV], FP32, isOutput=True)
    oaa = nc.declare_dram_parameter("acc_act", [P, NA], FP32, isOutput=True)
    with ExitStack() as es:
        ec = es.enter_context
        xt = ec(nc.sbuf_tensor([P, F], FP16))
        scr = ec(nc.sbuf_tensor([P, F], FP16))
        sact = ec(nc.sbuf_tensor([P, F], BF16))
        edt = ec(nc.sbuf_tensor([P, 2 * E], FP32))
        apk = ec(nc.sbuf_tensor([P, 2 * NV], FP32))
        aact = ec(nc.sbuf_tensor([P, NA], FP32))
        dse = ec(nc.semaphore("dse"))
        ds0 = ec(nc.semaphore("ds0"))
        ds1 = ec(nc.semaphore("ds1"))
        dout = ec(nc.semaphore("dout"))
        vsem = ec(nc.semaphore("vsem"))
        asem = ec(nc.semaphore("asem"))
        block = ec(nc.Block())

        @block.sync
        def _(sync):
            xv = x[:].rearrange("(p f) -> p f", p=P)
            sync.dma_start(edt[:], ed[:]).then_inc(dse, 16)
            sync.dma_start(xt[:, 0:H], xv[:, 0:H]).then_inc(ds0, 16)
            sync.dma_start(xt[:, H:F], xv[:, H:F]).then_inc(ds1, 16)
            # ACT finishes near the DVE; ship its (smaller) tile first.
            sync.wait_ge(asem, NA)
            sync.dma_start(oaa[:], aact[:]).then_inc(dout, 16)
            sync.wait_ge(vsem, 2 * NV)
            sync.dma_start(oad[:], apk[:]).then_inc(dout, 16)
            sync.wait_ge(dout, 32)

        @block.vector
        def _(vector):
            vector.wait_ge(dse, 16)
            vector.wait_ge(ds0, 16)
            for i, e_idx in enumerate(DVE_EDGES):
                vector.tensor_scalar(
                    scr[:, 0:H],
                    xt[:, 0:H],
                    edt[:, e_idx : e_idx + 1],
                    0.0,
                    OP.is_le,
                    op1=OP.add,
                    accum_out=apk[:, 2 * i : 2 * i + 1],
                ).then_inc(vsem, 1)
            vector.wait_ge(ds1, 16)
            for i, e_idx in enumerate(DVE_EDGES):
                vector.tensor_scalar(
                    scr[:, H:F],
                    xt[:, H:F],
                    edt[:, e_idx : e_idx + 1],
                    0.0,
                    OP.is_le,
                    op1=OP.add,
                    accum_out=apk[:, 2 * i + 1 : 2 * i + 2],
                ).then_inc(vsem, 1)

        @block.scalar
        def _(scalar):
            scalar.wait_ge(dse, 16)
            scalar.wait_ge(ds0, 16)
            scalar.wait_ge(ds1, 16)
            for j, e_idx in enumerate(ACT_EDGES):
                ne = edt[:, E + e_idx : E + e_idx + 1]  # negated edge
                scalar.activation(
                    sact[:], xt[:], ACT.Sign, bias=ne, scale=1.0,
                    accum_out=aact[:, j : j + 1],
                ).then_inc(asem, 1)
    return nc


def _build_pred(case: int):
    """Case-specialized predicate in fp16, chunked for DMA/compute overlap:
    0: x <= lo   1: x >= lo   2: (x >= lo) & (x <= up)   3: (x <= lo) | (x >= up)
    """
    nc = bass.Bass()
    x = nc.declare_dram_parameter("x", [DEV_N], FP16, isOutput=False)
    pr = nc.declare_dram_parameter("prm", [P, 8], FP32, isOutput=False)
    out = nc.declare_dram_parameter("pred", [DEV_N], FP16, isOutput=True)
    with ExitStack() as es:
        ec = es.enter_context
        xt = ec(nc.sbuf_tensor([P, F], FP16))
        tt = ec(nc.sbuf_tensor([P, F], FP16))
        po = ec(nc.sbuf_tensor([P, F], FP16))
        prm = ec(nc.sbuf_tensor([P, 8], FP32))
        dp = ec(nc.semaphore("dp"))
        dxq = [ec(nc.semaphore(f"dx{q}")) for q in range(4)]
        csem = ec(nc.semaphore("csem"))
        dout = ec(nc.semaphore("dout"))
        block = ec(nc.Block())

        @block.sync
        def _(sync):
            xv = x[:].rearrange("(p f) -> p f", p=P)
            ov = out[:].rearrange("(p f) -> p f", p=P)
            sync.dma_start(prm[:], pr[:]).then_inc(dp, 16)
            for q in range(4):
                sync.dma_start(
                    xt[:, q * Q : (q + 1) * Q], xv[:, q * Q : (q + 1) * Q]
                ).then_inc(dxq[q], 16)
            for q in range(4):
                sync.wait_ge(csem, q + 1)
                sync.dma_start(
                    ov[:, q * Q : (q + 1) * Q], po[:, q * Q : (q + 1) * Q]
                ).then_inc(dout, 16)
            sync.wait_ge(dout, 64)

        @block.vector
        def _(vector):
            vector.wait_ge(dp, 16)
            lo = prm[:, 0:1]
            up = prm[:, 1:2]
            for q in range(4):
                vector.wait_ge(dxq[q], 16)
                xs = xt[:, q * Q : (q + 1) * Q]
                ps = po[:, q * Q : (q + 1) * Q]
                ts = tt[:, q * Q : (q + 1) * Q]
                if case == 0:
                    vector.tensor_scalar(ps, xs, lo, None, OP.is_le).then_inc(
                        csem, 1
                    )
                elif case == 1:
                    vector.tensor_scalar(ps, xs, lo, None, OP.is_ge).then_inc(
                        csem, 1
                    )
                elif case == 2:
                    vector.tensor_scalar(ts, xs, up, None, OP.is_le)
                    vector.scalar_tensor_tensor(
                        ps, xs, lo, ts, op0=OP.is_ge, op1=OP.mult
                    ).then_inc(csem, 1)
                else:
                    vector.tensor_scalar(ts, xs, up, None, OP.is_ge)
                    vector.scalar_tensor_tensor(
                        ps, xs, lo, ts, op0=OP.is_le, op1=OP.add
                    ).then_inc(csem, 1)
    return nc


_PROGRAMS: dict = {}


def _prog(name):
    if name not in _PROGRAMS:
        if name.startswith("pred"):
            _PROGRAMS[name] = _build_pred(int(name[4:]))
        else:
            _PROGRAMS[name] = {
                "minmax": _build_minmax,
                "counts": _build_counts,
            }[name]()
    return _PROGRAMS[name]


# --------------------------------------------------------------------------
# Host orchestration
# --------------------------------------------------------------------------

LAST_EXEC_NS: list = []

_CACHE_SET = False


def _enable_jit_cache():
    global _CACHE_SET
    if _CACHE_SET:
        return
    _CACHE_SET = True
    try:
        import jax

        jax.config.update("jax_compilation_cache_dir", "/tmp/jax_bass_cache")
        jax.config.update("jax_persistent_cache_min_compile_time_secs", 1.0)
        jax.config.update("jax_persistent_cache_min_entry_size_bytes", 0)
    except Exception:
        pass


def _mock_one(name, m):
    if name == "minmax":
        v = m["x"].reshape(P, F)
        return {"mm": np.stack([v.min(axis=1), v.max(axis=1)], axis=1)}
    if name == "counts":
        v = m["x"].astype(np.float32).reshape(P, F)
        ed = m["edges"][0]
        apk = np.zeros((P, 2 * NV), np.float32)
        aact = np.zeros((P, NA), np.float32)
        for i, e_idx in enumerate(DVE_EDGES):
            e = ed[e_idx]
            apk[:, 2 * i] = (v[:, 0:H] <= e).sum(axis=1)
            apk[:, 2 * i + 1] = (v[:, H:F] <= e).sum(axis=1)
        for j, e_idx in enumerate(ACT_EDGES):
            e = ed[e_idx]
            aact[:, j] = np.sign(v - e).sum(axis=1)
        return {"acc_dve": apk, "acc_act": aact}
    if name.startswith("pred"):
        case = int(name[4:])
        v = m["x"].astype(np.float32)
        lo = m["prm"][0, 0]
        up = m["prm"][0, 1]
        if case == 0:
            p = v <= lo
        elif case == 1:
            p = v >= lo
        elif case == 2:
            p = (v >= lo) & (v <= up)
        else:
            p = (v <= lo) | (v >= up)
        return {"pred": p.astype(np.float16)}
    raise KeyError(name)


def _run(name, in_maps):
    _enable_jit_cache()
    if bool(int(os.environ.get("BASS_KERNEL_MOCK", "0"))):
        return [_mock_one(name, m) for m in in_maps]
    trace = bool(int(os.environ.get("BASS_KERNEL_PROFILE", "0")))
    r = run_bass_kernel_spmd(_prog(name), in_maps, CORE_IDS, trace=trace)
    if trace:
        LAST_EXEC_NS.append((name, r.exec_time_ns, r.mean_exec_time_ns))
    return r.results


def kernel(inputs: np.ndarray, targets: np.ndarray) -> np.ndarray:
    x_full = np.ascontiguousarray(inputs[:, 0]).astype(np.float32, copy=False)
    y_full = np.asarray(targets)
    assert x_full.shape[0] == N

    # ---- host prep: fp16 quantization + class compaction --------------------
    hdev_full = x_full.astype(np.float16)
    d_mask = np.abs(x_full) < F16_TINY  # fp16-subnormal guard
    hdev_full[d_mask] = np.float16(0.0)

    sig_idx = np.flatnonzero(y_full == 1)
    bkg_idx = np.flatnonzero(y_full != 1)
    perm = np.concatenate([sig_idx, bkg_idx])
    ns_cnt = int(sig_idx.size)
    assert ns_cnt <= N_DEV_TOT - F  # tail stays pure background

    xc = hdev_full[perm]                    # fp16, device order
    xc32 = xc.astype(np.float32)            # exact device-value replica
    xt_true = x_full[perm].astype(np.float64)
    d_c = d_mask[perm]

    shards = [
        np.ascontiguousarray(xc[c * DEV_N : (c + 1) * DEV_N]) for c in CORE_IDS
    ]

    # ---- L1: global min/max -------------------------------------------------
    LAST_EXEC_NS.clear()
    res1 = _run("minmax", [{"x": shards[c]} for c in CORE_IDS])
    hmm = np.stack([r["mm"] for r in res1])  # [8, P, 2] fp16
    hmin = np.float32(min(hmm[:, :, 0].min(), xc[N_DEV_TOT:].min()))
    hmax = np.float32(max(hmm[:, :, 1].max(), xc[N_DEV_TOT:].max()))
    gmin = np.float32(xt_true[xc32 == hmin].min())
    gmax = np.float32(xt_true[xc32 == hmax].max())
    # sentinel 0.0 (subnormal guard) must never be extremal
    assert gmin < -0.01 and gmax > 0.01

    # ---- edges: replicate jnp.linspace bit-exactly (eager CPU jax) ----------
    import jax
    import jax.numpy as jnp

    cpu = jax.devices("cpu")[0]
    with jax.default_device(cpu):
        edges = np.asarray(
            jnp.linspace(jnp.float32(gmin), jnp.float32(gmax), E)
        ).astype(np.float64)

    # ---- repair set: every element whose fp16 compare may disagree ----------
    h_step = (np.float64(gmax) - np.float64(gmin)) / N_BINS
    u = (xt_true - np.float64(gmin)) / h_step
    band = np.abs(u - np.rint(u)) < 0.02
    r_mask = band | d_c
    # fp16 rounding must stay well inside the band
    assert np.abs(xt_true - xc32)[~d_c].max() < 0.015 * h_step
    ridx = np.flatnonzero(r_mask)
    xr_true = xt_true[ridx]
    xr_dev = xc32[ridx].astype(np.float64)
    in_dev = ridx < N_DEV_TOT
    is_sig = ridx < ns_cnt

    TRU = xr_true[:, None] <= edges[None, :]   # [R, E]
    DEVP = xr_dev[:, None] <= edges[None, :]
    TIE = xr_true[:, None] == edges[None, :]

    delta_all = TRU[in_dev].sum(axis=0) - DEVP[in_dev].sum(axis=0)
    delta_sig = (
        TRU[in_dev & is_sig].sum(axis=0) - DEVP[in_dev & is_sig].sum(axis=0)
    )
    t_all = TIE.sum(axis=0).astype(np.float64)
    t_sig = TIE[is_sig].sum(axis=0).astype(np.float64)

    # device-value ties per (edge, chunk) for the ACT sign decode
    n_chunks = N_CORES * P
    eq_chunk = np.zeros((E, n_chunks), np.int64)
    rr, cc = np.nonzero((xr_dev[:, None] == edges[None, :]) & in_dev[:, None])
    if rr.size:
        np.add.at(eq_chunk, (cc, ridx[rr] // F), 1)

    # ---- L2: per-edge counts ------------------------------------------------
    ed_in = np.concatenate([edges, -edges]).astype(np.float32)
    edges_rep = np.ascontiguousarray(np.broadcast_to(ed_in, (P, 2 * E)))
    res2 = _run(
        "counts",
        [{"x": shards[c], "edges": edges_rep} for c in CORE_IDS],
    )

    # per-chunk le counts (device basis) for all 49 device edges
    le_chunk = np.zeros((E, n_chunks), np.float64)
    for c in CORE_IDS:
        a = res2[c]["acc_dve"].astype(np.float64)  # [P, 2*NV]
        s = res2[c]["acc_act"].astype(np.float64)  # [P, NA]
        cols = slice(c * P, (c + 1) * P)
        for i, e_idx in enumerate(DVE_EDGES):
            le_chunk[e_idx, cols] = a[:, 2 * i] + a[:, 2 * i + 1]
        for j, e_idx in enumerate(ACT_EDGES):
            le_chunk[e_idx, cols] = (
                F + eq_chunk[e_idx, cols] - s[:, j]
            ) / 2.0

    cnt_le = np.zeros(E, np.float64)
    sig_le = np.zeros(E, np.float64)
    dev_e = np.array(DEVE)

    cnt_le[dev_e] = le_chunk[dev_e].sum(axis=1)
    nfull = ns_cnt // F
    part = xc32[nfull * F : ns_cnt].astype(np.float64)
    sig_le[dev_e] = (
        le_chunk[dev_e, :nfull].sum(axis=1)
        + (part[:, None] <= edges[None, dev_e]).sum(axis=0)
    )

    # tail (pure background) + fp16-band repair to fp32 truth
    tail = xt_true[N_DEV_TOT:]
    cnt_le[dev_e] += (tail[:, None] <= edges[None, dev_e]).sum(axis=0)
    cnt_le[dev_e] += delta_all[dev_e]
    sig_le[dev_e] += delta_sig[dev_e]

    # edges 0 and 50 sit at the min/max: every element on the deciding side
    # is inside the repair band, so these counts are host-derived exactly
    # even if linspace's endpoints are off by an ulp.
    cnt_le[0] = TRU[:, 0].sum()
    sig_le[0] = TRU[is_sig, 0].sum()
    cnt_le[E - 1] = N - (len(ridx) - TRU[:, E - 1].sum())
    sig_le[E - 1] = ns_cnt - (int(is_sig.sum()) - TRU[is_sig, E - 1].sum())

    cnt_lt = cnt_le - t_all
    sig_lt = sig_le - t_sig

    ns_le = sig_le.astype(np.float32)
    ns_lt = sig_lt.astype(np.float32)
    nb_le = (cnt_le - sig_le).astype(np.float32)
    nb_lt = (cnt_lt - sig_lt).astype(np.float32)

    # ---- replicate the reference's tiny pair search (eager CPU jax) ---------
    with jax.default_device(cpu):
        ns_le_j = jnp.asarray(ns_le)
        ns_lt_j = jnp.asarray(ns_lt)
        nb_le_j = jnp.asarray(nb_le)
        nb_lt_j = jnp.asarray(nb_lt)
        n_f = jnp.float32(N)
        Ns = ns_le_j[-1]
        Nb = n_f - Ns

        hist0 = nb_le_j[1:] - nb_lt_j[:-1]
        hist1 = ns_le_j[1:] - ns_lt_j[:-1]

        gt0 = hist0 > hist1
        cand0 = jnp.logical_xor(gt0[:-1], gt0[1:]) & (hist0[:-1] > 0)
        gt1 = hist1 > hist0
        cand1 = jnp.logical_xor(gt1[:-1], gt1[1:]) & (hist1[:-1] > 0)
        mask = jnp.zeros((E,), bool).at[1:N_BINS].set(cand0 | cand1)
        cnt = jnp.sum(mask)
        mask = mask.at[-1].set(mask[-1] | (cnt == 1))

        a_c = -jnp.log1p(jnp.float32(-EPS))
        b_c = -jnp.log(jnp.float32(EPS))

        def bce(correct):
            return ((n_f - correct) * b_c + correct * a_c) / n_f

        c0 = ns_le_j + (Nb - nb_le_j)
        c1 = (Ns - ns_lt_j) + nb_lt_j
        c2 = (ns_le_j[None, :] - ns_lt_j[:, None]) + Nb - (
            nb_le_j[None, :] - nb_lt_j[:, None]
        )
        c3 = ns_le_j[:, None] + (Ns - ns_lt_j[None, :]) + (
            nb_le_j[None, :] - nb_lt_j[:, None]
        )

        L = jnp.stack(
            [
                jnp.broadcast_to(bce(c0)[:, None], (E, E)),
                jnp.broadcast_to(bce(c1)[:, None], (E, E)),
                bce(c2),
                bce(c3),
            ]
        )
        per_pair_min = jnp.min(L, axis=0)
        per_pair_case = jnp.argmin(L, axis=0)

        idxs = jnp.arange(E)
        valid = mask[:, None] & mask[None, :] & (idxs[:, None] < idxs[None, :])
        flat = jnp.argmin(jnp.where(valid, per_pair_min, jnp.inf))
        i = int(flat) // E
        j = int(flat) % E
        lower = np.float32(edges[i])
        upper = np.float32(edges[j])
        case = int(per_pair_case[i, j])

    # ---- L3: predicate ------------------------------------------------------
    prm = np.zeros((P, 8), np.float32)
    prm[:, 0] = lower
    prm[:, 1] = upper
    res3 = _run(
        f"pred{case}", [{"x": shards[c], "prm": prm} for c in CORE_IDS]
    )

    def true_pred(v):
        if case == 0:
            return v <= lower
        if case == 1:
            return v >= lower
        if case == 2:
            return (v >= lower) & (v <= upper)
        return (v <= lower) | (v >= upper)

    predc = np.empty(N, np.int32)
    predc[:N_DEV_TOT] = np.concatenate(
        [res3[c]["pred"] for c in CORE_IDS]
    ).astype(np.int32)
    predc[N_DEV_TOT:] = true_pred(tail).astype(np.int32)

    # patch every element whose fp16 compare vs lower/upper could disagree
    pband = 0.02 * h_step
    p_mask = (
        d_c
        | (np.abs(xt_true - np.float64(lower)) < pband)
        | (np.abs(xt_true - np.float64(upper)) < pband)
    )
    pidx = np.flatnonzero(p_mask)
    predc[pidx] = true_pred(xt_true[pidx]).astype(np.int32)

    out = np.empty(N, np.int32)
    out[perm] = predc
    return out


# revision 14
# speedup vs baseline: 1.5901x; 1.5887x over previous
"""Trainium2 Bass kernel for nn_CutLayer (histogram_binning) — v2.

Strategy (data-parallel over events, 8 cores):
  Host prep: extract feature column, quantize to fp16 (exact-repairable),
    class-compact globally (all signal events first, then background) so
    signal counts fall out of the per-partition accumulator structure.
  L1 minmax: per-partition min/max of the fp16 stream (DVE fold + reduce);
    host reconstructs the exact fp32 min/max from the tiny fp16-min bucket.
  L2 counts: per-edge cumulative counts via a single fp16 pass per edge:
    - DVE: tensor_scalar is_le with fp32 accumulator (4x perf mode).
    - ACT: Sign(x - e) with fp32 accumulator (1x, takes the leftover edges).
    Edges 0 and 50 equal the min/max and are host-derived; only 49 edges
    run on device. Counts are exact in fp16-space; the host repairs them
    to fp32-space using a small candidate band around each edge (all
    elements whose fp16 rounding could flip any compare), then runs the
    reference's tiny E^2 pair search bit-exactly with eager CPU jax.
  L3 pred: case-specialized predicate in fp16, chunked so the output DMA
    overlaps compute; host patches the band around the chosen thresholds
    and inverse-permutes back to event order.

Events: 8_000_000 total; device handles 8 * 128 * 7812 = 7_999_488 in
compacted order; the 512-element tail (all background) is host-exact.
"""

import os
from contextlib import ExitStack

import numpy as np

import concourse.bass as bass
import concourse.mybir as mybir
from concourse.bass_utils import run_bass_kernel_spmd

N = 8_000_000
N_CORES = 8
P = 128
F = 7812                         # free-dim columns per partition
H = F // 2
Q = F // 4
DEV_N = P * F                    # 999_936 device events per core
N_DEV_TOT = DEV_N * N_CORES      # 7_999_488
N_BINS = 50
E = N_BINS + 1                   # 51 edges
EPS = 1e-7
DEVE = list(range(1, E - 1))     # 49 device edges (0 and 50 host-derived)
# Edge split across engines. The DVE produces 0/1 indicator tiles at 4x rate
# for the PE edges (the PE sums them — DVE accumulate variants are 1x-only on
# this silicon); a few direct 1x-accumulate edges soak up leftover DVE time;
# the ACT engine takes the rest via the sign-sum trick.
NPE = 24
ND = 8
PE_EDGES = DEVE[:NPE]
DVE_EDGES = DEVE[NPE : NPE + ND]
ACT_EDGES = DEVE[NPE + ND :]
NV = len(DVE_EDGES)
NA = len(ACT_EDGES)
NBANK = 4                        # rotating PSUM banks for PE column sums
MM_SLICES = [(c, min(512, F - c)) for c in range(0, F, 512)]  # 15x512 + 132

FP32 = mybir.dt.float32
FP16 = mybir.dt.float16
BF16 = mybir.dt.bfloat16
AX = mybir.AxisListType
OP = mybir.AluOpType
ACT = mybir.ActivationFunctionType

CORE_IDS = list(range(N_CORES))

# fp16 min normal; |x| below this is routed through the host (sentinel 0.0
# on device) so fp16-subnormal flush behaviour can never matter.
F16_TINY = 6.2e-5


# --------------------------------------------------------------------------
# Bass programs
# --------------------------------------------------------------------------

def _build_minmax():
    nc = bass.Bass()
    x = nc.declare_dram_parameter("x", [DEV_N], FP16, isOutput=False)
    mm = nc.declare_dram_parameter("mm", [P, 2], FP16, isOutput=True)
    with (
        nc.sbuf_tensor([P, F], FP16) as xt,
        nc.sbuf_tensor([P, H], FP16) as s0,
        nc.sbuf_tensor([P, H], FP16) as s1,
        nc.sbuf_tensor([P, 2], FP16) as acc,
        nc.semaphore() as dsem,
        nc.semaphore() as csem,
        nc.Block() as block,
    ):
        @block.sync
        def _(sync):
            sync.dma_start(xt[:], x[:].rearrange("(p f) -> p f", p=P)).then_inc(
                dsem, 16
            )
            sync.wait_ge(csem, 4)
            sync.dma_start(mm[:], acc[:]).then_inc(dsem, 16)
            sync.wait_ge(dsem, 32)

        @block.vector
        def _(vector):
            vector.wait_ge(dsem, 16)
            vector.tensor_tensor(s0[:], xt[:, 0:H], xt[:, H:F], op=OP.min).then_inc(
                csem, 1
            )
            vector.tensor_reduce(acc[:, 0:1], s0[:], axis=AX.X, op=OP.min).then_inc(
                csem, 1
            )
            vector.tensor_tensor(s1[:], xt[:, 0:H], xt[:, H:F], op=OP.max).then_inc(
                csem, 1
            )
            vector.tensor_reduce(acc[:, 1:2], s1[:], axis=AX.X, op=OP.max).then_inc(
                csem, 1
            )
    return nc


def _build_counts():
    nc = bass.Bass()
    x = nc.declare_dram_parameter("x", [DEV_N], FP16, isOutput=False)
    ed = nc.declare_dram_parameter("edges", [P, 2 * E], FP32, isOutput=False)
    ones2 = nc.declare_dram_parameter("ones2", [P, 2], BF16, isOutput=False)
    ope = nc.declare_dram_parameter("acc_pe", [2 * NPE, 512], FP32, isOutput=True)
    oad = nc.declare_dram_parameter("acc_dve", [P, 2 * NV], FP32, isOutput=True)
    oaa = nc.declare_dram_parameter("acc_act", [P, NA], FP32, isOutput=True)
    with ExitStack() as es:
        ec = es.enter_context
        xt = ec(nc.sbuf_tensor([P, F], FP16))
        ind = [ec(nc.sbuf_tensor(f"ind{b}", [P, F], BF16)) for b in range(2)]
        scr = ec(nc.sbuf_tensor([P, F], FP16))
        sact = ec(nc.sbuf_tensor([P, F], BF16))
        edt = ec(nc.sbuf_tensor([P, 2 * E], FP32))
        o2t = ec(nc.sbuf_tensor([P, 2], BF16))
        apk = ec(nc.sbuf_tensor([P, 2 * NV], FP32))
        aact = ec(nc.sbuf_tensor([P, NA], FP32))
        nbk = (NPE + 2) // 3
        ps = [ec(nc.psum_tensor(f"ps{b}", [P, 512], FP32)) for b in range(nbk)]
        pcopy = ec(nc.sbuf_tensor("pcopy", [P, nbk * 512], FP32))
        dse = ec(nc.semaphore("dse"))
        ds0 = ec(nc.semaphore("ds0"))
        ds1 = ec(nc.semaphore("ds1"))
        do2 = ec(nc.semaphore("do2"))
        dout = ec(nc.semaphore("dout"))
        dpe = ec(nc.semaphore("dpe"))
        irdy = ec(nc.semaphore("irdy"))
        pdone = ec(nc.semaphore("pdone"))
        vsem = ec(nc.semaphore("vsem"))
        asem = ec(nc.semaphore("asem"))
        block = ec(nc.Block())

        @block.sync
        def _(sync):
            xv = x[:].rearrange("(p f) -> p f", p=P)
            sync.dma_start(edt[:], ed[:]).then_inc(dse, 16)
            sync.dma_start(o2t[:], ones2[:]).then_inc(do2, 16)
            sync.dma_start(xt[:, 0:H], xv[:, 0:H]).then_inc(ds0, 16)
            sync.dma_start(xt[:, H:F], xv[:, H:F]).then_inc(ds1, 16)
            sync.wait_ge(vsem, 2 * NV)
            sync.dma_start(oad[:], apk[:]).then_inc(dout, 16)
            sync.wait_ge(asem, NA)
            sync.dma_start(oaa[:], aact[:]).then_inc(dout, 16)
            for k in range(NPE):
                b, r = k // 3, 32 * (k % 3)
                sync.wait_ge(vsem, 2 * NV + b + 1)
                sync.dma_start(
                    ope[2 * k : 2 * k + 2],
                    pcopy[r : r + 2, b * 512 : (b + 1) * 512],
                ).then_inc(dpe, 16)
            sync.wait_ge(dout, 32)
            sync.wait_ge(dpe, 16 * NPE)

        @block.vector
        def _(vector):
            vector.wait_ge(dse, 16)
            vector.wait_ge(ds0, 16)
            vector.wait_ge(ds1, 16)
            # indicator tiles for the PE edges (4x rate), double-buffered
            for k, e_idx in enumerate(PE_EDGES):
                if k >= 2:
                    vector.wait_ge(pdone, k - 1)
                vector.tensor_scalar(
                    ind[k % 2][:],
                    xt[:],
                    edt[:, e_idx : e_idx + 1],
                    None,
                    OP.is_le,
                ).then_inc(irdy, 1)
            # leftover DVE time: direct 1x-accumulate edges
            for i, e_idx in enumerate(DVE_EDGES):
                for hh in range(2):
                    vector.tensor_scalar(
                        scr[:, hh * H : hh * H + H],
                        xt[:, hh * H : hh * H + H],
                        edt[:, e_idx : e_idx + 1],
                        0.0,
                        OP.is_le,
                        op1=OP.add,
                        accum_out=apk[:, 2 * i + hh : 2 * i + hh + 1],
                    ).then_inc(vsem, 1)
            # collect each PSUM bank (4 edges per bank) as it completes
            for b in range((NPE + 2) // 3):
                vector.wait_ge(pdone, min(3 * b + 3, NPE))
                vector.tensor_copy(
                    pcopy[:, b * 512 : (b + 1) * 512], ps[b][:]
                ).then_inc(vsem, 1)

        @block.tensor
        def _(tensor):
            tensor.wait_ge(do2, 16)
            for k in range(NPE):
                tensor.wait_ge(irdy, k + 1)
                src = ind[k % 2]
                bank, row = ps[k // 3], 32 * (k % 3)
                nsl = len(MM_SLICES)
                for s, (c0, w) in enumerate(MM_SLICES):
                    mm = tensor.matmul(
                        bank[row : row + 2, 0:w],
                        o2t[:],
                        src[:, c0 : c0 + w],
                        start=(s == 0),
                        stop=(s == nsl - 1),
                    )
                mm.then_inc(pdone, 1)

        @block.scalar
        def _(scalar):
            scalar.wait_ge(dse, 16)
            scalar.wait_ge(ds0, 16)
            scalar.wait_ge(ds1, 16)
            for j, e_idx in enumerate(ACT_EDGES):
                ne = edt[:, E + e_idx : E + e_idx + 1]  # negated edge
                scalar.activation(
                    sact[:], xt[:], ACT.Sign, bias=ne, scale=1.0,
                    accum_out=aact[:, j : j + 1],
                ).then_inc(asem, 1)
    return nc


def _build_pred(case: int):
    """Case-specialized predicate in fp16, chunked for DMA/compute overlap:
    0: x <= lo   1: x >= lo   2: (x >= lo) & (x <= up)   3: (x <= lo) | (x >= up)
    """
    nc = bass.Bass()
    x = nc.declare_dram_parameter("x", [DEV_N], FP16, isOutput=False)
    pr = nc.declare_dram_parameter("prm", [P, 8], FP32, isOutput=False)
    out = nc.declare_dram_parameter("pred", [DEV_N], FP16, isOutput=True)
    with ExitStack() as es:
        ec = es.enter_context
        xt = ec(nc.sbuf_tensor([P, F], FP16))
        tt = ec(nc.sbuf_tensor([P, F], FP16))
        po = ec(nc.sbuf_tensor([P, F], FP16))
        prm = ec(nc.sbuf_tensor([P, 8], FP32))
        dp = ec(nc.semaphore("dp"))
        dxq = [ec(nc.semaphore(f"dx{q}")) for q in range(4)]
        csem = ec(nc.semaphore("csem"))
        dout = ec(nc.semaphore("dout"))
        block = ec(nc.Block())

        @block.sync
        def _(sync):
            xv = x[:].rearrange("(p f) -> p f", p=P)
            ov = out[:].rearrange("(p f) -> p f", p=P)
            sync.dma_start(prm[:], pr[:]).then_inc(dp, 16)
            for q in range(4):
                sync.dma_start(
                    xt[:, q * Q : (q + 1) * Q], xv[:, q * Q : (q + 1) * Q]
                ).then_inc(dxq[q], 16)
            for q in range(4):
                sync.wait_ge(csem, q + 1)
                sync.dma_start(
                    ov[:, q * Q : (q + 1) * Q], po[:, q * Q : (q + 1) * Q]
                ).then_inc(dout, 16)
            sync.wait_ge(dout, 64)

        @block.vector
        def _(vector):
            vector.wait_ge(dp, 16)
            lo = prm[:, 0:1]
            up = prm[:, 1:2]
            for q in range(4):
                vector.wait_ge(dxq[q], 16)
                xs = xt[:, q * Q : (q + 1) * Q]
                ps = po[:, q * Q : (q + 1) * Q]
                ts = tt[:, q * Q : (q + 1) * Q]
                if case == 0:
                    vector.tensor_scalar(ps, xs, lo, None, OP.is_le).then_inc(
                        csem, 1
                    )
                elif case == 1:
                    vector.tensor_scalar(ps, xs, lo, None, OP.is_ge).then_inc(
                        csem, 1
                    )
                elif case == 2:
                    vector.tensor_scalar(ts, xs, up, None, OP.is_le)
                    vector.scalar_tensor_tensor(
                        ps, xs, lo, ts, op0=OP.is_ge, op1=OP.mult
                    ).then_inc(csem, 1)
                else:
                    vector.tensor_scalar(ts, xs, up, None, OP.is_ge)
                    vector.scalar_tensor_tensor(
                        ps, xs, lo, ts, op0=OP.is_le, op1=OP.add
                    ).then_inc(csem, 1)
    return nc


_PROGRAMS: dict = {}


def _prog(name):
    if name not in _PROGRAMS:
        if name.startswith("pred"):
            _PROGRAMS[name] = _build_pred(int(name[4:]))
        else:
            _PROGRAMS[name] = {
                "minmax": _build_minmax,
                "counts": _build_counts,
            }[name]()
    return _PROGRAMS[name]


# --------------------------------------------------------------------------
# Host orchestration
# --------------------------------------------------------------------------

LAST_EXEC_NS: list = []

_CACHE_SET = False


def _enable_jit_cache():
    global _CACHE_SET
    if _CACHE_SET:
        return
    _CACHE_SET = True
    try:
        import jax

        jax.config.update("jax_compilation_cache_dir", "/tmp/jax_bass_cache")
        jax.config.update("jax_persistent_cache_min_compile_time_secs", 1.0)
        jax.config.update("jax_persistent_cache_min_entry_size_bytes", 0)
    except Exception:
        pass


def _mock_one(name, m):
    if name == "minmax":
        v = m["x"].reshape(P, F)
        return {"mm": np.stack([v.min(axis=1), v.max(axis=1)], axis=1)}
    if name == "counts":
        v = m["x"].astype(np.float32).reshape(P, F)
        ed = m["edges"][0]
        o2 = m["ones2"].astype(np.float32)  # [P, 2]
        ope = np.zeros((2 * NPE, 512), np.float32)
        apk = np.zeros((P, 2 * NV), np.float32)
        aact = np.zeros((P, NA), np.float32)
        npad = len(MM_SLICES) * 512
        for k, e_idx in enumerate(PE_EDGES):
            indt = (v <= ed[e_idx]).astype(np.float32)
            cs = o2.T @ indt  # [2, F]
            csp = np.zeros((2, npad), np.float32)
            csp[:, :F] = cs
            ope[2 * k : 2 * k + 2] = csp.reshape(
                2, len(MM_SLICES), 512
            ).sum(axis=1)
        for i, e_idx in enumerate(DVE_EDGES):
            e = ed[e_idx]
            apk[:, 2 * i] = (v[:, 0:H] <= e).sum(axis=1)
            apk[:, 2 * i + 1] = (v[:, H:F] <= e).sum(axis=1)
        for j, e_idx in enumerate(ACT_EDGES):
            e = ed[e_idx]
            aact[:, j] = np.sign(v - e).sum(axis=1)
        return {"acc_pe": ope, "acc_dve": apk, "acc_act": aact}
    if name.startswith("pred"):
        case = int(name[4:])
        v = m["x"].astype(np.float32)
        lo = m["prm"][0, 0]
        up = m["prm"][0, 1]
        if case == 0:
            p = v <= lo
        elif case == 1:
            p = v >= lo
        elif case == 2:
            p = (v >= lo) & (v <= up)
        else:
            p = (v <= lo) | (v >= up)
        return {"pred": p.astype(np.float16)}
    raise KeyError(name)


def _run(name, in_maps):
    _enable_jit_cache()
    if bool(int(os.environ.get("BASS_KERNEL_MOCK", "0"))):
        return [_mock_one(name, m) for m in in_maps]
    trace = bool(int(os.environ.get("BASS_KERNEL_PROFILE", "0")))
    r = run_bass_kernel_spmd(_prog(name), in_maps, CORE_IDS, trace=trace)
    if trace:
        LAST_EXEC_NS.append((name, r.exec_time_ns, r.mean_exec_time_ns))
    return r.results


def kernel(inputs: np.ndarray, targets: np.ndarray) -> np.ndarray:
    x_full = np.ascontiguousarray(inputs[:, 0]).astype(np.float32, copy=False)
    y_full = np.asarray(targets)
    assert x_full.shape[0] == N

    # ---- host prep: fp16 quantization + class compaction --------------------
    hdev_full = x_full.astype(np.float16)
    d_mask = np.abs(x_full) < F16_TINY  # fp16-subnormal guard
    hdev_full[d_mask] = np.float16(0.0)

    sig_idx = np.flatnonzero(y_full == 1)
    bkg_idx = np.flatnonzero(y_full != 1)
    perm = np.concatenate([sig_idx, bkg_idx])
    ns_cnt = int(sig_idx.size)
    assert ns_cnt <= N_DEV_TOT - F  # tail stays pure background

    xc = hdev_full[perm]                    # fp16, device order
    xc32 = xc.astype(np.float32)            # exact device-value replica
    xt_true = x_full[perm].astype(np.float64)
    d_c = d_mask[perm]

    shards = [
        np.ascontiguousarray(xc[c * DEV_N : (c + 1) * DEV_N]) for c in CORE_IDS
    ]

    # ---- L1: global min/max -------------------------------------------------
    LAST_EXEC_NS.clear()
    res1 = _run("minmax", [{"x": shards[c]} for c in CORE_IDS])
    hmm = np.stack([r["mm"] for r in res1])  # [8, P, 2] fp16
    hmin = np.float32(min(hmm[:, :, 0].min(), xc[N_DEV_TOT:].min()))
    hmax = np.float32(max(hmm[:, :, 1].max(), xc[N_DEV_TOT:].max()))
    gmin = np.float32(xt_true[xc32 == hmin].min())
    gmax = np.float32(xt_true[xc32 == hmax].max())
    # sentinel 0.0 (subnormal guard) must never be extremal
    assert gmin < -0.01 and gmax > 0.01

    # ---- edges: replicate jnp.linspace bit-exactly (eager CPU jax) ----------
    import jax
    import jax.numpy as jnp

    cpu = jax.devices("cpu")[0]
    with jax.default_device(cpu):
        edges = np.asarray(
            jnp.linspace(jnp.float32(gmin), jnp.float32(gmax), E)
        ).astype(np.float64)

    # ---- repair set: every element whose fp16 compare may disagree ----------
    h_step = (np.float64(gmax) - np.float64(gmin)) / N_BINS
    u = (xt_true - np.float64(gmin)) / h_step
    band = np.abs(u - np.rint(u)) < 0.02
    r_mask = band | d_c
    # fp16 rounding must stay well inside the band
    assert np.abs(xt_true - xc32)[~d_c].max() < 0.015 * h_step
    ridx = np.flatnonzero(r_mask)
    xr_true = xt_true[ridx]
    xr_dev = xc32[ridx].astype(np.float64)
    in_dev = ridx < N_DEV_TOT
    is_sig = ridx < ns_cnt

    TRU = xr_true[:, None] <= edges[None, :]   # [R, E]
    DEVP = xr_dev[:, None] <= edges[None, :]
    TIE = xr_true[:, None] == edges[None, :]

    delta_all = TRU[in_dev].sum(axis=0) - DEVP[in_dev].sum(axis=0)
    delta_sig = (
        TRU[in_dev & is_sig].sum(axis=0) - DEVP[in_dev & is_sig].sum(axis=0)
    )
    t_all = TIE.sum(axis=0).astype(np.float64)
    t_sig = TIE[is_sig].sum(axis=0).astype(np.float64)

    # device-value ties per (edge, chunk) for the ACT sign decode
    n_chunks = N_CORES * P
    eq_chunk = np.zeros((E, n_chunks), np.int64)
    rr, cc = np.nonzero((xr_dev[:, None] == edges[None, :]) & in_dev[:, None])
    if rr.size:
        np.add.at(eq_chunk, (cc, ridx[rr] // F), 1)

    # ---- L2: per-edge counts ------------------------------------------------
    ed_in = np.concatenate([edges, -edges]).astype(np.float32)
    edges_rep = np.ascontiguousarray(np.broadcast_to(ed_in, (P, 2 * E)))
    nfull = ns_cnt // F
    ones2 = []
    for c in CORE_IDS:
        o2 = np.zeros((P, 2), np.float32)
        o2[:, 0] = 1.0
        gl = c * P + np.arange(P)
        o2[:, 1] = (gl < nfull).astype(np.float32)  # fully-signal chunks
        ones2.append(o2.astype(mybir.dt.np(BF16)))
    res2 = _run(
        "counts",
        [
            {"x": shards[c], "edges": edges_rep, "ones2": ones2[c]}
            for c in CORE_IDS
        ],
    )

    # device-basis totals and fully-signal-chunk subtotals per device edge
    tot_dev = np.zeros(E, np.float64)
    sigf_dev = np.zeros(E, np.float64)
    for c in CORE_IDS:
        pe = res2[c]["acc_pe"].astype(np.float64)  # [2*NPE, 512]
        a = res2[c]["acc_dve"].astype(np.float64)  # [P, 2*NV]
        s = res2[c]["acc_act"].astype(np.float64)  # [P, NA]
        cols = slice(c * P, (c + 1) * P)
        sig_mask = ones2[c][:, 1].astype(np.float64)
        for k, e_idx in enumerate(PE_EDGES):
            tot_dev[e_idx] += pe[2 * k].sum()
            sigf_dev[e_idx] += pe[2 * k + 1].sum()
        for i, e_idx in enumerate(DVE_EDGES):
            le_p = a[:, 2 * i] + a[:, 2 * i + 1]
            tot_dev[e_idx] += le_p.sum()
            sigf_dev[e_idx] += (le_p * sig_mask).sum()
        for j, e_idx in enumerate(ACT_EDGES):
            le_p = (F + eq_chunk[e_idx, cols] - s[:, j]) / 2.0
            tot_dev[e_idx] += le_p.sum()
            sigf_dev[e_idx] += (le_p * sig_mask).sum()

    cnt_le = np.zeros(E, np.float64)
    sig_le = np.zeros(E, np.float64)
    dev_e = np.array(DEVE)

    cnt_le[dev_e] = tot_dev[dev_e]
    part = xc32[nfull * F : ns_cnt].astype(np.float64)
    sig_le[dev_e] = sigf_dev[dev_e] + (
        part[:, None] <= edges[None, dev_e]
    ).sum(axis=0)

    # tail (pure background) + fp16-band repair to fp32 truth
    tail = xt_true[N_DEV_TOT:]
    cnt_le[dev_e] += (tail[:, None] <= edges[None, dev_e]).sum(axis=0)
    cnt_le[dev_e] += delta_all[dev_e]
    sig_le[dev_e] += delta_sig[dev_e]

    # edges 0 and 50 sit at the min/max: every element on the deciding side
    # is inside the repair band, so these counts are host-derived exactly
    # even if linspace's endpoints are off by an ulp.
    cnt_le[0] = TRU[:, 0].sum()
    sig_le[0] = TRU[is_sig, 0].sum()
    cnt_le[E - 1] = N - (len(ridx) - TRU[:, E - 1].sum())
    sig_le[E - 1] = ns_cnt - (int(is_sig.sum()) - TRU[is_sig, E - 1].sum())

    cnt_lt = cnt_le - t_all
    sig_lt = sig_le - t_sig

    ns_le = sig_le.astype(np.float32)
    ns_lt = sig_lt.astype(np.float32)
    nb_le = (cnt_le - sig_le).astype(np.float32)
    nb_lt = (cnt_lt - sig_lt).astype(np.float32)

    # ---- replicate the reference's tiny pair search (eager CPU jax) ---------
    with jax.default_device(cpu):
        ns_le_j = jnp.asarray(ns_le)
        ns_lt_j = jnp.asarray(ns_lt)
        nb_le_j = jnp.asarray(nb_le)
        nb_lt_j = jnp.asarray(nb_lt)
        n_f = jnp.float32(N)
        Ns = ns_le_j[-1]
        Nb = n_f - Ns

        hist0 = nb_le_j[1:] - nb_lt_j[:-1]
        hist1 = ns_le_j[1:] - ns_lt_j[:-1]

        gt0 = hist0 > hist1
        cand0 = jnp.logical_xor(gt0[:-1], gt0[1:]) & (hist0[:-1] > 0)
        gt1 = hist1 > hist0
        cand1 = jnp.logical_xor(gt1[:-1], gt1[1:]) & (hist1[:-1] > 0)
        mask = jnp.zeros((E,), bool).at[1:N_BINS].set(cand0 | cand1)
        cnt = jnp.sum(mask)
        mask = mask.at[-1].set(mask[-1] | (cnt == 1))

        a_c = -jnp.log1p(jnp.float32(-EPS))
        b_c = -jnp.log(jnp.float32(EPS))

        def bce(correct):
            return ((n_f - correct) * b_c + correct * a_c) / n_f

        c0 = ns_le_j + (Nb - nb_le_j)
        c1 = (Ns - ns_lt_j) + nb_lt_j
        c2 = (ns_le_j[None, :] - ns_lt_j[:, None]) + Nb - (
            nb_le_j[None, :] - nb_lt_j[:, None]
        )
        c3 = ns_le_j[:, None] + (Ns - ns_lt_j[None, :]) + (
            nb_le_j[None, :] - nb_lt_j[:, None]
        )

        L = jnp.stack(
            [
                jnp.broadcast_to(bce(c0)[:, None], (E, E)),
                jnp.broadcast_to(bce(c1)[:, None], (E, E)),
                bce(c2),
                bce(c3),
            ]
        )
        per_pair_min = jnp.min(L, axis=0)
        per_pair_case = jnp.argmin(L, axis=0)

        idxs = jnp.arange(E)
        valid = mask[:, None] & mask[None, :] & (idxs[:, None] < idxs[None, :])
        flat = jnp.argmin(jnp.where(valid, per_pair_min, jnp.inf))
        i = int(flat) // E
        j = int(flat) % E
        lower = np.float32(edges[i])
        upper = np.float32(edges[j])
        case = int(per_pair_case[i, j])

    # ---- L3: predicate ------------------------------------------------------
    prm = np.zeros((P, 8), np.float32)
    prm[:, 0] = lower
    prm[:, 1] = upper
    res3 = _run(
        f"pred{case}", [{"x": shards[c], "prm": prm} for c in CORE_IDS]
    )

    def true_pred(v):
        if case == 0:
            return v <= lower
        if case == 1:
            return v >= lower
        if case == 2:
            return (v >= lower) & (v <= upper)
        return (v <= lower) | (v >= upper)

    predc = np.empty(N, np.int32)
    predc[:N_DEV_TOT] = np.concatenate(
        [res3[c]["pred"] for c in CORE_IDS]
    ).astype(np.int32)
    predc[N_DEV_TOT:] = true_pred(tail).astype(np.int32)

    # patch every element whose fp16 compare vs lower/upper could disagree
    pband = 0.02 * h_step
    p_mask = (
        d_c
        | (np.abs(xt_true - np.float64(lower)) < pband)
        | (np.abs(xt_true - np.float64(upper)) < pband)
    )
    pidx = np.flatnonzero(p_mask)
    predc[pidx] = true_pred(xt_true[pidx]).astype(np.int32)

    out = np.empty(N, np.int32)
    out[perm] = predc
    return out


# revision 15
# speedup vs baseline: 1.9068x; 1.1992x over previous
"""Trainium2 Bass kernel for nn_CutLayer (histogram_binning) — v2.

Strategy (data-parallel over events, 8 cores):
  Host prep: extract feature column, quantize to fp16 (exact-repairable),
    class-compact globally (all signal events first, then background) so
    signal counts fall out of the per-partition accumulator structure.
  L1 minmax: per-partition min/max of the fp16 stream (DVE fold + reduce);
    host reconstructs the exact fp32 min/max from the tiny fp16-min bucket.
  L2 counts: per-edge cumulative counts via a single fp16 pass per edge:
    - DVE: tensor_scalar is_le with fp32 accumulator (4x perf mode).
    - ACT: Sign(x - e) with fp32 accumulator (1x, takes the leftover edges).
    Edges 0 and 50 equal the min/max and are host-derived; only 49 edges
    run on device. Counts are exact in fp16-space; the host repairs them
    to fp32-space using a small candidate band around each edge (all
    elements whose fp16 rounding could flip any compare), then runs the
    reference's tiny E^2 pair search bit-exactly with eager CPU jax.
  L3 pred: case-specialized predicate in fp16, chunked so the output DMA
    overlaps compute; host patches the band around the chosen thresholds
    and inverse-permutes back to event order.

Events: 8_000_000 total; device handles 8 * 128 * 7812 = 7_999_488 in
compacted order; the 512-element tail (all background) is host-exact.
"""

import os
from contextlib import ExitStack

import numpy as np

import concourse.bass as bass
import concourse.mybir as mybir
from concourse.bass_utils import run_bass_kernel_spmd

N = 8_000_000
N_CORES = 8
P = 128
F = 7812                         # free-dim columns per partition
H = F // 2
Q = F // 4
DEV_N = P * F                    # 999_936 device events per core
N_DEV_TOT = DEV_N * N_CORES      # 7_999_488
N_BINS = 50
E = N_BINS + 1                   # 51 edges
EPS = 1e-7
DEVE = list(range(1, E - 1))     # 49 device edges (0 and 50 host-derived)
# Edge split across engines. The DVE produces 0/1 indicator tiles at 4x rate
# for the PE edges (the PE sums them — DVE accumulate variants are 1x-only on
# this silicon); a few direct 1x-accumulate edges soak up leftover DVE time;
# the ACT engine takes the rest via the sign-sum trick.
NPE = 33
ND = 2
PE_EDGES = DEVE[:NPE]
DVE_EDGES = DEVE[NPE : NPE + ND]
ACT_EDGES = DEVE[NPE + ND :]
NV = len(DVE_EDGES)
NA = len(ACT_EDGES)
NBUF = 6                         # DVE->PE indicator run-ahead buffers
MM_SLICES = [(c, min(512, F - c)) for c in range(0, F, 512)]  # 15x512 + 132

FP32 = mybir.dt.float32
FP16 = mybir.dt.float16
BF16 = mybir.dt.bfloat16
AX = mybir.AxisListType
OP = mybir.AluOpType
ACT = mybir.ActivationFunctionType

CORE_IDS = list(range(N_CORES))

# fp16 min normal; |x| below this is routed through the host (sentinel 0.0
# on device) so fp16-subnormal flush behaviour can never matter.
F16_TINY = 6.2e-5


# --------------------------------------------------------------------------
# Bass programs
# --------------------------------------------------------------------------

def _build_minmax():
    nc = bass.Bass()
    x = nc.declare_dram_parameter("x", [DEV_N], FP16, isOutput=False)
    mm = nc.declare_dram_parameter("mm", [P, 2], FP16, isOutput=True)
    with (
        nc.sbuf_tensor([P, F], FP16) as xt,
        nc.sbuf_tensor([P, H], FP16) as s0,
        nc.sbuf_tensor([P, H], FP16) as s1,
        nc.sbuf_tensor([P, 2], FP16) as acc,
        nc.semaphore() as dsem,
        nc.semaphore() as csem,
        nc.Block() as block,
    ):
        @block.sync
        def _(sync):
            sync.dma_start(xt[:], x[:].rearrange("(p f) -> p f", p=P)).then_inc(
                dsem, 16
            )
            sync.wait_ge(csem, 4)
            sync.dma_start(mm[:], acc[:]).then_inc(dsem, 16)
            sync.wait_ge(dsem, 32)

        @block.vector
        def _(vector):
            vector.wait_ge(dsem, 16)
            vector.tensor_tensor(s0[:], xt[:, 0:H], xt[:, H:F], op=OP.min).then_inc(
                csem, 1
            )
            vector.tensor_reduce(acc[:, 0:1], s0[:], axis=AX.X, op=OP.min).then_inc(
                csem, 1
            )
            vector.tensor_tensor(s1[:], xt[:, 0:H], xt[:, H:F], op=OP.max).then_inc(
                csem, 1
            )
            vector.tensor_reduce(acc[:, 1:2], s1[:], axis=AX.X, op=OP.max).then_inc(
                csem, 1
            )
    return nc


def _build_counts():
    nc = bass.Bass()
    x = nc.declare_dram_parameter("x", [DEV_N], FP16, isOutput=False)
    ed = nc.declare_dram_parameter("edges", [P, 2 * E], FP32, isOutput=False)
    ones2 = nc.declare_dram_parameter("ones2", [P, 2], BF16, isOutput=False)
    ope = nc.declare_dram_parameter("acc_pe", [2 * NPE, 512], FP32, isOutput=True)
    oad = nc.declare_dram_parameter("acc_dve", [P, 2 * NV], FP32, isOutput=True)
    oaa = nc.declare_dram_parameter("acc_act", [P, NA], FP32, isOutput=True)
    with ExitStack() as es:
        ec = es.enter_context
        xt = ec(nc.sbuf_tensor([P, F], FP16))
        ind = [ec(nc.sbuf_tensor(f"ind{b}", [P, F], BF16)) for b in range(NBUF)]
        scr = ec(nc.sbuf_tensor([P, F], FP16))
        sact = ec(nc.sbuf_tensor([P, F], BF16))
        edt = ec(nc.sbuf_tensor([P, 2 * E], FP32))
        o2t = ec(nc.sbuf_tensor([P, 2], BF16))
        apk = ec(nc.sbuf_tensor([P, 2 * NV], FP32))
        aact = ec(nc.sbuf_tensor([P, NA], FP32))
        nslot = (NPE + 2) // 3
        ps = [ec(nc.psum_tensor(f"ps{b}", [P, 512], FP32)) for b in range(8)]
        pcopy = ec(nc.sbuf_tensor("pcopy", [P, nslot * 512], FP32))
        cps = ec(nc.semaphore("cps"))
        dse = ec(nc.semaphore("dse"))
        ds0 = ec(nc.semaphore("ds0"))
        ds1 = ec(nc.semaphore("ds1"))
        do2 = ec(nc.semaphore("do2"))
        dout = ec(nc.semaphore("dout"))
        dpe = ec(nc.semaphore("dpe"))
        irdy = ec(nc.semaphore("irdy"))
        pdone = ec(nc.semaphore("pdone"))
        vsem = ec(nc.semaphore("vsem"))
        asem = ec(nc.semaphore("asem"))
        block = ec(nc.Block())

        @block.sync
        def _(sync):
            xv = x[:].rearrange("(p f) -> p f", p=P)
            sync.dma_start(edt[:], ed[:]).then_inc(dse, 16)
            sync.dma_start(o2t[:], ones2[:]).then_inc(do2, 16)
            sync.dma_start(xt[:, 0:H], xv[:, 0:H]).then_inc(ds0, 16)
            sync.dma_start(xt[:, H:F], xv[:, H:F]).then_inc(ds1, 16)
            sync.wait_ge(vsem, 2 * NV)
            sync.dma_start(oad[:], apk[:]).then_inc(dout, 16)
            sync.wait_ge(asem, NA)
            sync.dma_start(oaa[:], aact[:]).then_inc(dout, 16)
            for k in range(NPE):
                b, r = k // 3, 32 * (k % 3)
                sync.wait_ge(cps, b + 1)
                sync.dma_start(
                    ope[2 * k : 2 * k + 2],
                    pcopy[r : r + 2, b * 512 : (b + 1) * 512],
                ).then_inc(dpe, 16)
            sync.wait_ge(dout, 32)
            sync.wait_ge(dpe, 16 * NPE)

        @block.vector
        def _(vector):
            vector.wait_ge(dse, 16)
            vector.wait_ge(ds0, 16)
            vector.wait_ge(ds1, 16)
            # The PE consumes indicators slower than the DVE produces them
            # (~3.1us vs ~2.25us per edge), so the direct-accumulate edges
            # and the PSUM bank copies are statically interleaved into the
            # indicator stream as filler for the run-ahead budget (NBUF).
            nslot = (NPE + 2) // 3
            direct_halves = [
                (i, e_idx, hh)
                for i, e_idx in enumerate(DVE_EDGES)
                for hh in range(2)
            ]
            dh_at = {10: 0, 16: 1, 22: 2, 28: 3}
            ncopy = 0

            def do_copy(vector, c):
                vector.wait_ge(pdone, min(3 * c + 3, NPE))
                vector.tensor_copy(
                    pcopy[:, c * 512 : (c + 1) * 512], ps[c % 8][:]
                ).then_inc(cps, 1)

            for k, e_idx in enumerate(PE_EDGES):
                if k >= NBUF:
                    vector.wait_ge(pdone, k - (NBUF - 1))
                vector.tensor_scalar(
                    ind[k % NBUF][:],
                    xt[:],
                    edt[:, e_idx : e_idx + 1],
                    None,
                    OP.is_le,
                ).then_inc(irdy, 1)
                if k >= 8 and (k - 8) % 3 == 0 and ncopy < 8:
                    do_copy(vector, ncopy)
                    ncopy += 1
                if k in dh_at and dh_at[k] < len(direct_halves):
                    i, de_idx, hh = direct_halves[dh_at[k]]
                    vector.tensor_scalar(
                        scr[:, hh * H : hh * H + H],
                        xt[:, hh * H : hh * H + H],
                        edt[:, de_idx : de_idx + 1],
                        0.0,
                        OP.is_le,
                        op1=OP.add,
                        accum_out=apk[:, 2 * i + hh : 2 * i + hh + 1],
                    ).then_inc(vsem, 1)
            for dh in range(len(dh_at), len(direct_halves)):
                i, de_idx, hh = direct_halves[dh]
                vector.tensor_scalar(
                    scr[:, hh * H : hh * H + H],
                    xt[:, hh * H : hh * H + H],
                    edt[:, de_idx : de_idx + 1],
                    0.0,
                    OP.is_le,
                    op1=OP.add,
                    accum_out=apk[:, 2 * i + hh : 2 * i + hh + 1],
                ).then_inc(vsem, 1)
            while ncopy < nslot:
                do_copy(vector, ncopy)
                ncopy += 1

        @block.tensor
        def _(tensor):
            tensor.wait_ge(do2, 16)
            for k in range(NPE):
                tensor.wait_ge(irdy, k + 1)
                if k // 3 >= 8:
                    tensor.wait_ge(cps, k // 3 - 7)
                src = ind[k % NBUF]
                bank, row = ps[(k // 3) % 8], 32 * (k % 3)
                nsl = len(MM_SLICES)
                for s, (c0, w) in enumerate(MM_SLICES):
                    mm = tensor.matmul(
                        bank[row : row + 2, 0:w],
                        o2t[:],
                        src[:, c0 : c0 + w],
                        start=(s == 0),
                        stop=(s == nsl - 1),
                    )
                mm.then_inc(pdone, 1)

        @block.scalar
        def _(scalar):
            scalar.wait_ge(dse, 16)
            scalar.wait_ge(ds0, 16)
            scalar.wait_ge(ds1, 16)
            for j, e_idx in enumerate(ACT_EDGES):
                ne = edt[:, E + e_idx : E + e_idx + 1]  # negated edge
                scalar.activation(
                    sact[:], xt[:], ACT.Sign, bias=ne, scale=1.0,
                    accum_out=aact[:, j : j + 1],
                ).then_inc(asem, 1)
    return nc


def _build_pred(case: int):
    """Case-specialized predicate in fp16, chunked for DMA/compute overlap:
    0: x <= lo   1: x >= lo   2: (x >= lo) & (x <= up)   3: (x <= lo) | (x >= up)
    """
    nc = bass.Bass()
    x = nc.declare_dram_parameter("x", [DEV_N], FP16, isOutput=False)
    pr = nc.declare_dram_parameter("prm", [P, 8], FP32, isOutput=False)
    out = nc.declare_dram_parameter("pred", [DEV_N], FP16, isOutput=True)
    with ExitStack() as es:
        ec = es.enter_context
        xt = ec(nc.sbuf_tensor([P, F], FP16))
        tt = ec(nc.sbuf_tensor([P, F], FP16))
        po = ec(nc.sbuf_tensor([P, F], FP16))
        prm = ec(nc.sbuf_tensor([P, 8], FP32))
        dp = ec(nc.semaphore("dp"))
        dxq = [ec(nc.semaphore(f"dx{q}")) for q in range(4)]
        csem = ec(nc.semaphore("csem"))
        dout = ec(nc.semaphore("dout"))
        block = ec(nc.Block())

        @block.sync
        def _(sync):
            xv = x[:].rearrange("(p f) -> p f", p=P)
            ov = out[:].rearrange("(p f) -> p f", p=P)
            sync.dma_start(prm[:], pr[:]).then_inc(dp, 16)
            for q in range(4):
                sync.dma_start(
                    xt[:, q * Q : (q + 1) * Q], xv[:, q * Q : (q + 1) * Q]
                ).then_inc(dxq[q], 16)
            for q in range(4):
                sync.wait_ge(csem, q + 1)
                sync.dma_start(
                    ov[:, q * Q : (q + 1) * Q], po[:, q * Q : (q + 1) * Q]
                ).then_inc(dout, 16)
            sync.wait_ge(dout, 64)

        @block.vector
        def _(vector):
            vector.wait_ge(dp, 16)
            lo = prm[:, 0:1]
            up = prm[:, 1:2]
            for q in range(4):
                vector.wait_ge(dxq[q], 16)
                xs = xt[:, q * Q : (q + 1) * Q]
                ps = po[:, q * Q : (q + 1) * Q]
                ts = tt[:, q * Q : (q + 1) * Q]
                if case == 0:
                    vector.tensor_scalar(ps, xs, lo, None, OP.is_le).then_inc(
                        csem, 1
                    )
                elif case == 1:
                    vector.tensor_scalar(ps, xs, lo, None, OP.is_ge).then_inc(
                        csem, 1
                    )
                elif case == 2:
                    vector.tensor_scalar(ts, xs, up, None, OP.is_le)
                    vector.scalar_tensor_tensor(
                        ps, xs, lo, ts, op0=OP.is_ge, op1=OP.mult
                    ).then_inc(csem, 1)
                else:
                    vector.tensor_scalar(ts, xs, up, None, OP.is_ge)
                    vector.scalar_tensor_tensor(
                        ps, xs, lo, ts, op0=OP.is_le, op1=OP.add
                    ).then_inc(csem, 1)
    return nc


_PROGRAMS: dict = {}


def _prog(name):
    if name not in _PROGRAMS:
        if name.startswith("pred"):
            _PROGRAMS[name] = _build_pred(int(name[4:]))
        else:
            _PROGRAMS[name] = {
                "minmax": _build_minmax,
                "counts": _build_counts,
            }[name]()
    return _PROGRAMS[name]


# --------------------------------------------------------------------------
# Host orchestration
# --------------------------------------------------------------------------

LAST_EXEC_NS: list = []

_CACHE_SET = False


def _enable_jit_cache():
    global _CACHE_SET
    if _CACHE_SET:
        return
    _CACHE_SET = True
    try:
        import jax

        jax.config.update("jax_compilation_cache_dir", "/tmp/jax_bass_cache")
        jax.config.update("jax_persistent_cache_min_compile_time_secs", 1.0)
        jax.config.update("jax_persistent_cache_min_entry_size_bytes", 0)
    except Exception:
        pass


def _mock_one(name, m):
    if name == "minmax":
        v = m["x"].reshape(P, F)
        return {"mm": np.stack([v.min(axis=1), v.max(axis=1)], axis=1)}
    if name == "counts":
        v = m["x"].astype(np.float32).reshape(P, F)
        ed = m["edges"][0]
        o2 = m["ones2"].astype(np.float32)  # [P, 2]
        ope = np.zeros((2 * NPE, 512), np.float32)
        apk = np.zeros((P, 2 * NV), np.float32)
        aact = np.zeros((P, NA), np.float32)
        npad = len(MM_SLICES) * 512
        for k, e_idx in enumerate(PE_EDGES):
            indt = (v <= ed[e_idx]).astype(np.float32)
            cs = o2.T @ indt  # [2, F]
            csp = np.zeros((2, npad), np.float32)
            csp[:, :F] = cs
            ope[2 * k : 2 * k + 2] = csp.reshape(
                2, len(MM_SLICES), 512
            ).sum(axis=1)
        for i, e_idx in enumerate(DVE_EDGES):
            e = ed[e_idx]
            apk[:, 2 * i] = (v[:, 0:H] <= e).sum(axis=1)
            apk[:, 2 * i + 1] = (v[:, H:F] <= e).sum(axis=1)
        for j, e_idx in enumerate(ACT_EDGES):
            e = ed[e_idx]
            aact[:, j] = np.sign(v - e).sum(axis=1)
        return {"acc_pe": ope, "acc_dve": apk, "acc_act": aact}
    if name.startswith("pred"):
        case = int(name[4:])
        v = m["x"].astype(np.float32)
        lo = m["prm"][0, 0]
        up = m["prm"][0, 1]
        if case == 0:
            p = v <= lo
        elif case == 1:
            p = v >= lo
        elif case == 2:
            p = (v >= lo) & (v <= up)
        else:
            p = (v <= lo) | (v >= up)
        return {"pred": p.astype(np.float16)}
    raise KeyError(name)


def _run(name, in_maps):
    _enable_jit_cache()
    if bool(int(os.environ.get("BASS_KERNEL_MOCK", "0"))):
        return [_mock_one(name, m) for m in in_maps]
    trace = bool(int(os.environ.get("BASS_KERNEL_PROFILE", "0")))
    r = run_bass_kernel_spmd(_prog(name), in_maps, CORE_IDS, trace=trace)
    if trace:
        LAST_EXEC_NS.append((name, r.exec_time_ns, r.mean_exec_time_ns))
    return r.results


def kernel(inputs: np.ndarray, targets: np.ndarray) -> np.ndarray:
    x_full = np.ascontiguousarray(inputs[:, 0]).astype(np.float32, copy=False)
    y_full = np.asarray(targets)
    assert x_full.shape[0] == N

    # ---- host prep: fp16 quantization + class compaction --------------------
    hdev_full = x_full.astype(np.float16)
    d_mask = np.abs(x_full) < F16_TINY  # fp16-subnormal guard
    hdev_full[d_mask] = np.float16(0.0)

    sig_idx = np.flatnonzero(y_full == 1)
    bkg_idx = np.flatnonzero(y_full != 1)
    perm = np.concatenate([sig_idx, bkg_idx])
    ns_cnt = int(sig_idx.size)
    assert ns_cnt <= N_DEV_TOT - F  # tail stays pure background

    xc = hdev_full[perm]                    # fp16, device order
    xc32 = xc.astype(np.float32)            # exact device-value replica
    xt_true = x_full[perm].astype(np.float64)
    d_c = d_mask[perm]

    shards = [
        np.ascontiguousarray(xc[c * DEV_N : (c + 1) * DEV_N]) for c in CORE_IDS
    ]

    # ---- L1: global min/max -------------------------------------------------
    LAST_EXEC_NS.clear()
    res1 = _run("minmax", [{"x": shards[c]} for c in CORE_IDS])
    hmm = np.stack([r["mm"] for r in res1])  # [8, P, 2] fp16
    hmin = np.float32(min(hmm[:, :, 0].min(), xc[N_DEV_TOT:].min()))
    hmax = np.float32(max(hmm[:, :, 1].max(), xc[N_DEV_TOT:].max()))
    gmin = np.float32(xt_true[xc32 == hmin].min())
    gmax = np.float32(xt_true[xc32 == hmax].max())
    # sentinel 0.0 (subnormal guard) must never be extremal
    assert gmin < -0.01 and gmax > 0.01

    # ---- edges: replicate jnp.linspace bit-exactly (eager CPU jax) ----------
    import jax
    import jax.numpy as jnp

    cpu = jax.devices("cpu")[0]
    with jax.default_device(cpu):
        edges = np.asarray(
            jnp.linspace(jnp.float32(gmin), jnp.float32(gmax), E)
        ).astype(np.float64)

    # ---- repair set: every element whose fp16 compare may disagree ----------
    h_step = (np.float64(gmax) - np.float64(gmin)) / N_BINS
    u = (xt_true - np.float64(gmin)) / h_step
    band = np.abs(u - np.rint(u)) < 0.02
    r_mask = band | d_c
    # fp16 rounding must stay well inside the band
    assert np.abs(xt_true - xc32)[~d_c].max() < 0.015 * h_step
    ridx = np.flatnonzero(r_mask)
    xr_true = xt_true[ridx]
    xr_dev = xc32[ridx].astype(np.float64)
    in_dev = ridx < N_DEV_TOT
    is_sig = ridx < ns_cnt

    TRU = xr_true[:, None] <= edges[None, :]   # [R, E]
    DEVP = xr_dev[:, None] <= edges[None, :]
    TIE = xr_true[:, None] == edges[None, :]

    delta_all = TRU[in_dev].sum(axis=0) - DEVP[in_dev].sum(axis=0)
    delta_sig = (
        TRU[in_dev & is_sig].sum(axis=0) - DEVP[in_dev & is_sig].sum(axis=0)
    )
    t_all = TIE.sum(axis=0).astype(np.float64)
    t_sig = TIE[is_sig].sum(axis=0).astype(np.float64)

    # device-value ties per (edge, chunk) for the ACT sign decode
    n_chunks = N_CORES * P
    eq_chunk = np.zeros((E, n_chunks), np.int64)
    rr, cc = np.nonzero((xr_dev[:, None] == edges[None, :]) & in_dev[:, None])
    if rr.size:
        np.add.at(eq_chunk, (cc, ridx[rr] // F), 1)

    # ---- L2: per-edge counts ------------------------------------------------
    ed_in = np.concatenate([edges, -edges]).astype(np.float32)
    edges_rep = np.ascontiguousarray(np.broadcast_to(ed_in, (P, 2 * E)))
    nfull = ns_cnt // F
    ones2 = []
    for c in CORE_IDS:
        o2 = np.zeros((P, 2), np.float32)
        o2[:, 0] = 1.0
        gl = c * P + np.arange(P)
        o2[:, 1] = (gl < nfull).astype(np.float32)  # fully-signal chunks
        ones2.append(o2.astype(mybir.dt.np(BF16)))
    res2 = _run(
        "counts",
        [
            {"x": shards[c], "edges": edges_rep, "ones2": ones2[c]}
            for c in CORE_IDS
        ],
    )

    # device-basis totals and fully-signal-chunk subtotals per device edge
    tot_dev = np.zeros(E, np.float64)
    sigf_dev = np.zeros(E, np.float64)
    for c in CORE_IDS:
        pe = res2[c]["acc_pe"].astype(np.float64)  # [2*NPE, 512]
        a = res2[c]["acc_dve"].astype(np.float64)  # [P, 2*NV]
        s = res2[c]["acc_act"].astype(np.float64)  # [P, NA]
        cols = slice(c * P, (c + 1) * P)
        sig_mask = ones2[c][:, 1].astype(np.float64)
        for k, e_idx in enumerate(PE_EDGES):
            tot_dev[e_idx] += pe[2 * k].sum()
            sigf_dev[e_idx] += pe[2 * k + 1].sum()
        for i, e_idx in enumerate(DVE_EDGES):
            le_p = a[:, 2 * i] + a[:, 2 * i + 1]
            tot_dev[e_idx] += le_p.sum()
            sigf_dev[e_idx] += (le_p * sig_mask).sum()
        for j, e_idx in enumerate(ACT_EDGES):
            le_p = (F + eq_chunk[e_idx, cols] - s[:, j]) / 2.0
            tot_dev[e_idx] += le_p.sum()
            sigf_dev[e_idx] += (le_p * sig_mask).sum()

    cnt_le = np.zeros(E, np.float64)
    sig_le = np.zeros(E, np.float64)
    dev_e = np.array(DEVE)

    cnt_le[dev_e] = tot_dev[dev_e]
    part = xc32[nfull * F : ns_cnt].astype(np.float64)
    sig_le[dev_e] = sigf_dev[dev_e] + (
        part[:, None] <= edges[None, dev_e]
    ).sum(axis=0)

    # tail (pure background) + fp16-band repair to fp32 truth
    tail = xt_true[N_DEV_TOT:]
    cnt_le[dev_e] += (tail[:, None] <= edges[None, dev_e]).sum(axis=0)
    cnt_le[dev_e] += delta_all[dev_e]
    sig_le[dev_e] += delta_sig[dev_e]

    # edges 0 and 50 sit at the min/max: every element on the deciding side
    # is inside the repair band, so these counts are host-derived exactly
    # even if linspace's endpoints are off by an ulp.
    cnt_le[0] = TRU[:, 0].sum()
    sig_le[0] = TRU[is_sig, 0].sum()
    cnt_le[E - 1] = N - (len(ridx) - TRU[:, E - 1].sum())
    sig_le[E - 1] = ns_cnt - (int(is_sig.sum()) - TRU[is_sig, E - 1].sum())

    cnt_lt = cnt_le - t_all
    sig_lt = sig_le - t_sig

    ns_le = sig_le.astype(np.float32)
    ns_lt = sig_lt.astype(np.float32)
    nb_le = (cnt_le - sig_le).astype(np.float32)
    nb_lt = (cnt_lt - sig_lt).astype(np.float32)

    # ---- replicate the reference's tiny pair search (eager CPU jax) ---------
    with jax.default_device(cpu):
        ns_le_j = jnp.asarray(ns_le)
        ns_lt_j = jnp.asarray(ns_lt)
        nb_le_j = jnp.asarray(nb_le)
        nb_lt_j = jnp.asarray(nb_lt)
        n_f = jnp.float32(N)
        Ns = ns_le_j[-1]
        Nb = n_f - Ns

        hist0 = nb_le_j[1:] - nb_lt_j[:-1]
        hist1 = ns_le_j[1:] - ns_lt_j[:-1]

        gt0 = hist0 > hist1
        cand0 = jnp.logical_xor(gt0[:-1], gt0[1:]) & (hist0[:-1] > 0)
        gt1 = hist1 > hist0
        cand1 = jnp.logical_xor(gt1[:-1], gt1[1:]) & (hist1[:-1] > 0)
        mask = jnp.zeros((E,), bool).at[1:N_BINS].set(cand0 | cand1)
        cnt = jnp.sum(mask)
        mask = mask.at[-1].set(mask[-1] | (cnt == 1))

        a_c = -jnp.log1p(jnp.float32(-EPS))
        b_c = -jnp.log(jnp.float32(EPS))

        def bce(correct):
            return ((n_f - correct) * b_c + correct * a_c) / n_f

        c0 = ns_le_j + (Nb - nb_le_j)
        c1 = (Ns - ns_lt_j) + nb_lt_j
        c2 = (ns_le_j[None, :] - ns_lt_j[:, None]) + Nb - (
            nb_le_j[None, :] - nb_lt_j[:, None]
        )
        c3 = ns_le_j[:, None] + (Ns - ns_lt_j[None, :]) + (
            nb_le_j[None, :] - nb_lt_j[:, None]
        )

        L = jnp.stack(
            [
                jnp.broadcast_to(bce(c0)[:, None], (E, E)),
                jnp.broadcast_to(bce(c1)[:, None], (E, E)),
                bce(c2),
                bce(c3),
            ]
        )
        per_pair_min = jnp.min(L, axis=0)
        per_pair_case = jnp.argmin(L, axis=0)

        idxs = jnp.arange(E)
        valid = mask[:, None] & mask[None, :] & (idxs[:, None] < idxs[None, :])
        flat = jnp.argmin(jnp.where(valid, per_pair_min, jnp.inf))
        i = int(flat) // E
        j = int(flat) % E
        lower = np.float32(edges[i])
        upper = np.float32(edges[j])
        case = int(per_pair_case[i, j])

    # ---- L3: predicate ------------------------------------------------------
    prm = np.zeros((P, 8), np.float32)
    prm[:, 0] = lower
    prm[:, 1] = upper
    res3 = _run(
        f"pred{case}", [{"x": shards[c], "prm": prm} for c in CORE_IDS]
    )

    def true_pred(v):
        if case == 0:
            return v <= lower
        if case == 1:
            return v >= lower
        if case == 2:
            return (v >= lower) & (v <= upper)
        return (v <= lower) | (v >= upper)

    predc = np.empty(N, np.int32)
    predc[:N_DEV_TOT] = np.concatenate(
        [res3[c]["pred"] for c in CORE_IDS]
    ).astype(np.int32)
    predc[N_DEV_TOT:] = true_pred(tail).astype(np.int32)

    # patch every element whose fp16 compare vs lower/upper could disagree
    pband = 0.02 * h_step
    p_mask = (
        d_c
        | (np.abs(xt_true - np.float64(lower)) < pband)
        | (np.abs(xt_true - np.float64(upper)) < pband)
    )
    pidx = np.flatnonzero(p_mask)
    predc[pidx] = true_pred(xt_true[pidx]).astype(np.int32)

    out = np.empty(N, np.int32)
    out[perm] = predc
    return out


# revision 16
# speedup vs baseline: 2.1140x; 1.1086x over previous
"""Trainium2 Bass kernel for nn_CutLayer (histogram_binning) — v2.

Strategy (data-parallel over events, 8 cores):
  Host prep: extract feature column, quantize to fp16 (exact-repairable),
    class-compact globally (all signal events first, then background) so
    signal counts fall out of the per-partition accumulator structure.
  L1 minmax: per-partition min/max of the fp16 stream (DVE fold + reduce);
    host reconstructs the exact fp32 min/max from the tiny fp16-min bucket.
  L2 counts: per-edge cumulative counts via a single fp16 pass per edge:
    - DVE: tensor_scalar is_le with fp32 accumulator (4x perf mode).
    - ACT: Sign(x - e) with fp32 accumulator (1x, takes the leftover edges).
    Edges 0 and 50 equal the min/max and are host-derived; only 49 edges
    run on device. Counts are exact in fp16-space; the host repairs them
    to fp32-space using a small candidate band around each edge (all
    elements whose fp16 rounding could flip any compare), then runs the
    reference's tiny E^2 pair search bit-exactly with eager CPU jax.
  L3 pred: case-specialized predicate in fp16, chunked so the output DMA
    overlaps compute; host patches the band around the chosen thresholds
    and inverse-permutes back to event order.

Events: 8_000_000 total; device handles 8 * 128 * 7812 = 7_999_488 in
compacted order; the 512-element tail (all background) is host-exact.
"""

import os
from contextlib import ExitStack

import numpy as np

import concourse.bass as bass
import concourse.mybir as mybir
from concourse.bass_utils import run_bass_kernel_spmd

N = 8_000_000
N_CORES = 8
P = 128
F = 7812                         # free-dim columns per partition
H = F // 2
Q = F // 4
DEV_N = P * F                    # 999_936 device events per core
N_DEV_TOT = DEV_N * N_CORES      # 7_999_488
N_BINS = 50
E = N_BINS + 1                   # 51 edges
EPS = 1e-7
DEVE = list(range(1, E - 1))     # 49 device edges (0 and 50 host-derived)
# Edge split across engines. The DVE produces 0/1 indicator tiles at 4x rate
# for the PE edges (the PE sums them — DVE accumulate variants are 1x-only on
# this silicon); a few direct 1x-accumulate edges soak up leftover DVE time;
# the ACT engine takes the rest via the sign-sum trick.
NPE = 33
ND = 2
PE_EDGES = DEVE[:NPE]
DVE_EDGES = DEVE[NPE : NPE + ND]
ACT_EDGES = DEVE[NPE + ND :]
NV = len(DVE_EDGES)
NA = len(ACT_EDGES)
NBUF = 6                         # DVE->PE indicator run-ahead buffers
MM_SLICES = [(c, min(512, F - c)) for c in range(0, F, 512)]  # 15x512 + 132

FP32 = mybir.dt.float32
FP16 = mybir.dt.float16
BF16 = mybir.dt.bfloat16
AX = mybir.AxisListType
OP = mybir.AluOpType
ACT = mybir.ActivationFunctionType

CORE_IDS = list(range(N_CORES))

# fp16 min normal; |x| below this is routed through the host (sentinel 0.0
# on device) so fp16-subnormal flush behaviour can never matter.
F16_TINY = 6.2e-5


# --------------------------------------------------------------------------
# Bass programs
# --------------------------------------------------------------------------

def _build_minmax():
    nc = bass.Bass()
    x = nc.declare_dram_parameter("x", [DEV_N], FP16, isOutput=False)
    mm = nc.declare_dram_parameter("mm", [P, 2], FP16, isOutput=True)
    with (
        nc.sbuf_tensor([P, F], FP16) as xt,
        nc.sbuf_tensor([P, Q], FP16) as m0,
        nc.sbuf_tensor([P, Q], FP16) as m1,
        nc.sbuf_tensor([P, Q], FP16) as x0,
        nc.sbuf_tensor([P, Q], FP16) as x1,
        nc.sbuf_tensor([P, 2], FP16) as acc,
        nc.semaphore() as ds0,
        nc.semaphore() as ds1,
        nc.semaphore() as csem,
        nc.semaphore() as dsem,
        nc.Block() as block,
    ):
        @block.sync
        def _(sync):
            xv = x[:].rearrange("(p f) -> p f", p=P)
            sync.dma_start(xt[:, 0:H], xv[:, 0:H]).then_inc(ds0, 16)
            sync.dma_start(xt[:, H:F], xv[:, H:F]).then_inc(ds1, 16)
            sync.wait_ge(csem, 8)
            sync.dma_start(mm[:], acc[:]).then_inc(dsem, 16)
            sync.wait_ge(dsem, 16)

        @block.vector
        def _(vector):
            # fold each half into quarters while the other half streams in
            vector.wait_ge(ds0, 16)
            vector.tensor_tensor(m0[:], xt[:, 0:Q], xt[:, Q:H], op=OP.min).then_inc(
                csem, 1
            )
            vector.tensor_tensor(x0[:], xt[:, 0:Q], xt[:, Q:H], op=OP.max).then_inc(
                csem, 1
            )
            vector.wait_ge(ds1, 16)
            vector.tensor_tensor(
                m1[:], xt[:, H : H + Q], xt[:, H + Q : F], op=OP.min
            ).then_inc(csem, 1)
            vector.tensor_tensor(
                x1[:], xt[:, H : H + Q], xt[:, H + Q : F], op=OP.max
            ).then_inc(csem, 1)
            vector.tensor_tensor(m0[:], m0[:], m1[:], op=OP.min).then_inc(csem, 1)
            vector.tensor_tensor(x0[:], x0[:], x1[:], op=OP.max).then_inc(csem, 1)
            vector.tensor_reduce(acc[:, 0:1], m0[:], axis=AX.X, op=OP.min).then_inc(
                csem, 1
            )
            vector.tensor_reduce(acc[:, 1:2], x0[:], axis=AX.X, op=OP.max).then_inc(
                csem, 1
            )
    return nc


def _build_counts():
    nc = bass.Bass()
    x = nc.declare_dram_parameter("x", [DEV_N], FP16, isOutput=False)
    ed = nc.declare_dram_parameter("edges", [P, 2 * E], FP32, isOutput=False)
    ones2 = nc.declare_dram_parameter("ones2", [P, 2], BF16, isOutput=False)
    NSLOT = (NPE + 2) // 3
    ope = nc.declare_dram_parameter(
        "acc_pe", [3, 2, NSLOT * 512], FP32, isOutput=True
    )
    oad = nc.declare_dram_parameter("acc_dve", [P, 2 * NV], FP32, isOutput=True)
    oaa = nc.declare_dram_parameter("acc_act", [P, NA], FP32, isOutput=True)
    with ExitStack() as es:
        ec = es.enter_context
        xt = ec(nc.sbuf_tensor([P, F], FP16))
        ind = [ec(nc.sbuf_tensor(f"ind{b}", [P, F], BF16)) for b in range(NBUF)]
        scr = ec(nc.sbuf_tensor([P, F], FP16))
        sact = ec(nc.sbuf_tensor([P, F], BF16))
        edt = ec(nc.sbuf_tensor([P, 2 * E], FP32))
        o2t = ec(nc.sbuf_tensor([P, 2], BF16))
        apk = ec(nc.sbuf_tensor([P, 2 * NV], FP32))
        aact = ec(nc.sbuf_tensor([P, NA], FP32))
        nslot = (NPE + 2) // 3
        ps = [ec(nc.psum_tensor(f"ps{b}", [P, 512], FP32)) for b in range(8)]
        pcopy = ec(nc.sbuf_tensor("pcopy", [P, nslot * 512], FP32))
        cps = ec(nc.semaphore("cps"))
        dse = ec(nc.semaphore("dse"))
        ds0 = ec(nc.semaphore("ds0"))
        ds1 = ec(nc.semaphore("ds1"))
        do2 = ec(nc.semaphore("do2"))
        dout = ec(nc.semaphore("dout"))
        dpe = ec(nc.semaphore("dpe"))
        irdy = ec(nc.semaphore("irdy"))
        pdone = ec(nc.semaphore("pdone"))
        vsem = ec(nc.semaphore("vsem"))
        asem = ec(nc.semaphore("asem"))
        block = ec(nc.Block())

        @block.sync
        def _(sync):
            xv = x[:].rearrange("(p f) -> p f", p=P)
            sync.dma_start(xt[:, 0:H], xv[:, 0:H]).then_inc(ds0, 16)
            sync.dma_start(xt[:, H:F], xv[:, H:F]).then_inc(ds1, 16)
            sync.dma_start(edt[:], ed[:]).then_inc(dse, 16)
            sync.dma_start(o2t[:], ones2[:]).then_inc(do2, 16)
            sync.wait_ge(vsem, 2 * NV)
            sync.dma_start(oad[:], apk[:]).then_inc(dout, 16)
            sync.wait_ge(asem, NA)
            sync.dma_start(oaa[:], aact[:]).then_inc(dout, 16)
            sync.wait_ge(cps, NSLOT)
            for r in range(3):
                sync.dma_start(
                    ope[r], pcopy[32 * r : 32 * r + 2, :]
                ).then_inc(dpe, 16)
            sync.wait_ge(dout, 32)
            sync.wait_ge(dpe, 48)

        @block.vector
        def _(vector):
            vector.wait_ge(dse, 16)
            vector.wait_ge(ds0, 16)
            vector.wait_ge(ds1, 16)
            # The PE consumes indicators slower than the DVE produces them
            # (~3.1us vs ~2.25us per edge), so the direct-accumulate edges
            # and the PSUM bank copies are statically interleaved into the
            # indicator stream as filler for the run-ahead budget (NBUF).
            nslot = (NPE + 2) // 3
            direct_halves = [
                (i, e_idx, hh)
                for i, e_idx in enumerate(DVE_EDGES)
                for hh in range(2)
            ]
            dh_at = {10: 0, 16: 1, 22: 2, 28: 3}
            ncopy = 0

            def do_copy(vector, c):
                vector.wait_ge(pdone, min(3 * c + 3, NPE))
                vector.tensor_copy(
                    pcopy[:, c * 512 : (c + 1) * 512], ps[c % 8][:]
                ).then_inc(cps, 1)

            for k, e_idx in enumerate(PE_EDGES):
                if k >= NBUF:
                    vector.wait_ge(pdone, k - (NBUF - 1))
                vector.tensor_scalar(
                    ind[k % NBUF][:],
                    xt[:],
                    edt[:, e_idx : e_idx + 1],
                    None,
                    OP.is_le,
                ).then_inc(irdy, 1)
                if k >= 8 and (k - 8) % 3 == 0 and ncopy < 8:
                    do_copy(vector, ncopy)
                    ncopy += 1
                if k in dh_at and dh_at[k] < len(direct_halves):
                    i, de_idx, hh = direct_halves[dh_at[k]]
                    vector.tensor_scalar(
                        scr[:, hh * H : hh * H + H],
                        xt[:, hh * H : hh * H + H],
                        edt[:, de_idx : de_idx + 1],
                        0.0,
                        OP.is_le,
                        op1=OP.add,
                        accum_out=apk[:, 2 * i + hh : 2 * i + hh + 1],
                    ).then_inc(vsem, 1)
            for dh in range(len(dh_at), len(direct_halves)):
                i, de_idx, hh = direct_halves[dh]
                vector.tensor_scalar(
                    scr[:, hh * H : hh * H + H],
                    xt[:, hh * H : hh * H + H],
                    edt[:, de_idx : de_idx + 1],
                    0.0,
                    OP.is_le,
                    op1=OP.add,
                    accum_out=apk[:, 2 * i + hh : 2 * i + hh + 1],
                ).then_inc(vsem, 1)
            while ncopy < nslot:
                do_copy(vector, ncopy)
                ncopy += 1

        @block.tensor
        def _(tensor):
            tensor.wait_ge(do2, 16)
            for k in range(NPE):
                tensor.wait_ge(irdy, k + 1)
                if k // 3 >= 8:
                    tensor.wait_ge(cps, k // 3 - 7)
                src = ind[k % NBUF]
                bank, row = ps[(k // 3) % 8], 32 * (k % 3)
                nsl = len(MM_SLICES)
                for s, (c0, w) in enumerate(MM_SLICES):
                    mm = tensor.matmul(
                        bank[row : row + 2, 0:w],
                        o2t[:],
                        src[:, c0 : c0 + w],
                        start=(s == 0),
                        stop=(s == nsl - 1),
                    )
                mm.then_inc(pdone, 1)

        @block.scalar
        def _(scalar):
            scalar.wait_ge(dse, 16)
            scalar.wait_ge(ds0, 16)
            scalar.wait_ge(ds1, 16)
            for j, e_idx in enumerate(ACT_EDGES):
                ne = edt[:, E + e_idx : E + e_idx + 1]  # negated edge
                scalar.activation(
                    sact[:], xt[:], ACT.Sign, bias=ne, scale=1.0,
                    accum_out=aact[:, j : j + 1],
                ).then_inc(asem, 1)
    return nc


def _build_pred(case: int):
    """Case-specialized predicate in fp16, chunked for DMA/compute overlap:
    0: x <= lo   1: x >= lo   2: (x >= lo) & (x <= up)   3: (x <= lo) | (x >= up)
    """
    nc = bass.Bass()
    x = nc.declare_dram_parameter("x", [DEV_N], FP16, isOutput=False)
    pr = nc.declare_dram_parameter("prm", [P, 8], FP32, isOutput=False)
    out = nc.declare_dram_parameter("pred", [DEV_N], FP16, isOutput=True)
    with ExitStack() as es:
        ec = es.enter_context
        xt = ec(nc.sbuf_tensor([P, F], FP16))
        tt = ec(nc.sbuf_tensor([P, F], FP16))
        po = ec(nc.sbuf_tensor([P, F], FP16))
        prm = ec(nc.sbuf_tensor([P, 8], FP32))
        dp = ec(nc.semaphore("dp"))
        dxq = [ec(nc.semaphore(f"dx{q}")) for q in range(4)]
        csem = ec(nc.semaphore("csem"))
        dout = ec(nc.semaphore("dout"))
        block = ec(nc.Block())

        @block.sync
        def _(sync):
            xv = x[:].rearrange("(p f) -> p f", p=P)
            ov = out[:].rearrange("(p f) -> p f", p=P)
            sync.dma_start(prm[:], pr[:]).then_inc(dp, 16)
            for q in range(4):
                sync.dma_start(
                    xt[:, q * Q : (q + 1) * Q], xv[:, q * Q : (q + 1) * Q]
                ).then_inc(dxq[q], 16)
            for q in range(4):
                sync.wait_ge(csem, q + 1)
                sync.dma_start(
                    ov[:, q * Q : (q + 1) * Q], po[:, q * Q : (q + 1) * Q]
                ).then_inc(dout, 16)
            sync.wait_ge(dout, 64)

        @block.vector
        def _(vector):
            vector.wait_ge(dp, 16)
            lo = prm[:, 0:1]
            up = prm[:, 1:2]
            for q in range(4):
                vector.wait_ge(dxq[q], 16)
                xs = xt[:, q * Q : (q + 1) * Q]
                ps = po[:, q * Q : (q + 1) * Q]
                ts = tt[:, q * Q : (q + 1) * Q]
                if case == 0:
                    vector.tensor_scalar(ps, xs, lo, None, OP.is_le).then_inc(
                        csem, 1
                    )
                elif case == 1:
                    vector.tensor_scalar(ps, xs, lo, None, OP.is_ge).then_inc(
                        csem, 1
                    )
                elif case == 2:
                    vector.tensor_scalar(ts, xs, up, None, OP.is_le)
                    vector.scalar_tensor_tensor(
                        ps, xs, lo, ts, op0=OP.is_ge, op1=OP.mult
                    ).then_inc(csem, 1)
                else:
                    vector.tensor_scalar(ts, xs, up, None, OP.is_ge)
                    vector.scalar_tensor_tensor(
                        ps, xs, lo, ts, op0=OP.is_le, op1=OP.add
                    ).then_inc(csem, 1)
    return nc


_PROGRAMS: dict = {}


def _prog(name):
    if name not in _PROGRAMS:
        if name.startswith("pred"):
            _PROGRAMS[name] = _build_pred(int(name[4:]))
        else:
            _PROGRAMS[name] = {
                "minmax": _build_minmax,
                "counts": _build_counts,
            }[name]()
    return _PROGRAMS[name]


# --------------------------------------------------------------------------
# Host orchestration
# --------------------------------------------------------------------------

LAST_EXEC_NS: list = []

_CACHE_SET = False


def _enable_jit_cache():
    global _CACHE_SET
    if _CACHE_SET:
        return
    _CACHE_SET = True
    try:
        import jax

        jax.config.update("jax_compilation_cache_dir", "/tmp/jax_bass_cache")
        jax.config.update("jax_persistent_cache_min_compile_time_secs", 1.0)
        jax.config.update("jax_persistent_cache_min_entry_size_bytes", 0)
    except Exception:
        pass


def _mock_one(name, m):
    if name == "minmax":
        v = m["x"].reshape(P, F)
        return {"mm": np.stack([v.min(axis=1), v.max(axis=1)], axis=1)}
    if name == "counts":
        v = m["x"].astype(np.float32).reshape(P, F)
        ed = m["edges"][0]
        o2 = m["ones2"].astype(np.float32)  # [P, 2]
        nslot = (NPE + 2) // 3
        ope = np.zeros((3, 2, nslot * 512), np.float32)
        apk = np.zeros((P, 2 * NV), np.float32)
        aact = np.zeros((P, NA), np.float32)
        npad = len(MM_SLICES) * 512
        for k, e_idx in enumerate(PE_EDGES):
            indt = (v <= ed[e_idx]).astype(np.float32)
            cs = o2.T @ indt  # [2, F]
            csp = np.zeros((2, npad), np.float32)
            csp[:, :F] = cs
            ope[k % 3, :, (k // 3) * 512 : (k // 3) * 512 + 512] = (
                csp.reshape(2, len(MM_SLICES), 512).sum(axis=1)
            )
        for i, e_idx in enumerate(DVE_EDGES):
            e = ed[e_idx]
            apk[:, 2 * i] = (v[:, 0:H] <= e).sum(axis=1)
            apk[:, 2 * i + 1] = (v[:, H:F] <= e).sum(axis=1)
        for j, e_idx in enumerate(ACT_EDGES):
            e = ed[e_idx]
            aact[:, j] = np.sign(v - e).sum(axis=1)
        return {"acc_pe": ope, "acc_dve": apk, "acc_act": aact}
    if name.startswith("pred"):
        case = int(name[4:])
        v = m["x"].astype(np.float32)
        lo = m["prm"][0, 0]
        up = m["prm"][0, 1]
        if case == 0:
            p = v <= lo
        elif case == 1:
            p = v >= lo
        elif case == 2:
            p = (v >= lo) & (v <= up)
        else:
            p = (v <= lo) | (v >= up)
        return {"pred": p.astype(np.float16)}
    raise KeyError(name)


def _run(name, in_maps):
    _enable_jit_cache()
    if bool(int(os.environ.get("BASS_KERNEL_MOCK", "0"))):
        return [_mock_one(name, m) for m in in_maps]
    trace = bool(int(os.environ.get("BASS_KERNEL_PROFILE", "0")))
    r = run_bass_kernel_spmd(_prog(name), in_maps, CORE_IDS, trace=trace)
    if trace:
        LAST_EXEC_NS.append((name, r.exec_time_ns, r.mean_exec_time_ns))
    return r.results


def kernel(inputs: np.ndarray, targets: np.ndarray) -> np.ndarray:
    x_full = np.ascontiguousarray(inputs[:, 0]).astype(np.float32, copy=False)
    y_full = np.asarray(targets)
    assert x_full.shape[0] == N

    # ---- host prep: fp16 quantization + class compaction --------------------
    hdev_full = x_full.astype(np.float16)
    d_mask = np.abs(x_full) < F16_TINY  # fp16-subnormal guard
    hdev_full[d_mask] = np.float16(0.0)

    sig_idx = np.flatnonzero(y_full == 1)
    bkg_idx = np.flatnonzero(y_full != 1)
    perm = np.concatenate([sig_idx, bkg_idx])
    ns_cnt = int(sig_idx.size)
    assert ns_cnt <= N_DEV_TOT - F  # tail stays pure background

    xc = hdev_full[perm]                    # fp16, device order
    xc32 = xc.astype(np.float32)            # exact device-value replica
    xt_true = x_full[perm].astype(np.float64)
    d_c = d_mask[perm]

    shards = [
        np.ascontiguousarray(xc[c * DEV_N : (c + 1) * DEV_N]) for c in CORE_IDS
    ]

    # ---- L1: global min/max -------------------------------------------------
    LAST_EXEC_NS.clear()
    res1 = _run("minmax", [{"x": shards[c]} for c in CORE_IDS])
    hmm = np.stack([r["mm"] for r in res1])  # [8, P, 2] fp16
    hmin = np.float32(min(hmm[:, :, 0].min(), xc[N_DEV_TOT:].min()))
    hmax = np.float32(max(hmm[:, :, 1].max(), xc[N_DEV_TOT:].max()))
    gmin = np.float32(xt_true[xc32 == hmin].min())
    gmax = np.float32(xt_true[xc32 == hmax].max())
    # sentinel 0.0 (subnormal guard) must never be extremal
    assert gmin < -0.01 and gmax > 0.01

    # ---- edges: replicate jnp.linspace bit-exactly (eager CPU jax) ----------
    import jax
    import jax.numpy as jnp

    cpu = jax.devices("cpu")[0]
    with jax.default_device(cpu):
        edges = np.asarray(
            jnp.linspace(jnp.float32(gmin), jnp.float32(gmax), E)
        ).astype(np.float64)

    # ---- repair set: every element whose fp16 compare may disagree ----------
    h_step = (np.float64(gmax) - np.float64(gmin)) / N_BINS
    u = (xt_true - np.float64(gmin)) / h_step
    band = np.abs(u - np.rint(u)) < 0.02
    r_mask = band | d_c
    # fp16 rounding must stay well inside the band
    assert np.abs(xt_true - xc32)[~d_c].max() < 0.015 * h_step
    ridx = np.flatnonzero(r_mask)
    xr_true = xt_true[ridx]
    xr_dev = xc32[ridx].astype(np.float64)
    in_dev = ridx < N_DEV_TOT
    is_sig = ridx < ns_cnt

    TRU = xr_true[:, None] <= edges[None, :]   # [R, E]
    DEVP = xr_dev[:, None] <= edges[None, :]
    TIE = xr_true[:, None] == edges[None, :]

    delta_all = TRU[in_dev].sum(axis=0) - DEVP[in_dev].sum(axis=0)
    delta_sig = (
        TRU[in_dev & is_sig].sum(axis=0) - DEVP[in_dev & is_sig].sum(axis=0)
    )
    t_all = TIE.sum(axis=0).astype(np.float64)
    t_sig = TIE[is_sig].sum(axis=0).astype(np.float64)

    # device-value ties per (edge, chunk) for the ACT sign decode
    n_chunks = N_CORES * P
    eq_chunk = np.zeros((E, n_chunks), np.int64)
    rr, cc = np.nonzero((xr_dev[:, None] == edges[None, :]) & in_dev[:, None])
    if rr.size:
        np.add.at(eq_chunk, (cc, ridx[rr] // F), 1)

    # ---- L2: per-edge counts ------------------------------------------------
    ed_in = np.concatenate([edges, -edges]).astype(np.float32)
    edges_rep = np.ascontiguousarray(np.broadcast_to(ed_in, (P, 2 * E)))
    nfull = ns_cnt // F
    ones2 = []
    for c in CORE_IDS:
        o2 = np.zeros((P, 2), np.float32)
        o2[:, 0] = 1.0
        gl = c * P + np.arange(P)
        o2[:, 1] = (gl < nfull).astype(np.float32)  # fully-signal chunks
        ones2.append(o2.astype(mybir.dt.np(BF16)))
    res2 = _run(
        "counts",
        [
            {"x": shards[c], "edges": edges_rep, "ones2": ones2[c]}
            for c in CORE_IDS
        ],
    )

    # device-basis totals and fully-signal-chunk subtotals per device edge
    tot_dev = np.zeros(E, np.float64)
    sigf_dev = np.zeros(E, np.float64)
    for c in CORE_IDS:
        pe = res2[c]["acc_pe"].astype(np.float64)  # [3, 2, NSLOT*512]
        a = res2[c]["acc_dve"].astype(np.float64)  # [P, 2*NV]
        s = res2[c]["acc_act"].astype(np.float64)  # [P, NA]
        cols = slice(c * P, (c + 1) * P)
        sig_mask = ones2[c][:, 1].astype(np.float64)
        for k, e_idx in enumerate(PE_EDGES):
            pcols = slice((k // 3) * 512, (k // 3) * 512 + 512)
            tot_dev[e_idx] += pe[k % 3, 0, pcols].sum()
            sigf_dev[e_idx] += pe[k % 3, 1, pcols].sum()
        for i, e_idx in enumerate(DVE_EDGES):
            le_p = a[:, 2 * i] + a[:, 2 * i + 1]
            tot_dev[e_idx] += le_p.sum()
            sigf_dev[e_idx] += (le_p * sig_mask).sum()
        for j, e_idx in enumerate(ACT_EDGES):
            le_p = (F + eq_chunk[e_idx, cols] - s[:, j]) / 2.0
            tot_dev[e_idx] += le_p.sum()
            sigf_dev[e_idx] += (le_p * sig_mask).sum()

    cnt_le = np.zeros(E, np.float64)
    sig_le = np.zeros(E, np.float64)
    dev_e = np.array(DEVE)

    cnt_le[dev_e] = tot_dev[dev_e]
    part = xc32[nfull * F : ns_cnt].astype(np.float64)
    sig_le[dev_e] = sigf_dev[dev_e] + (
        part[:, None] <= edges[None, dev_e]
    ).sum(axis=0)

    # tail (pure background) + fp16-band repair to fp32 truth
    tail = xt_true[N_DEV_TOT:]
    cnt_le[dev_e] += (tail[:, None] <= edges[None, dev_e]).sum(axis=0)
    cnt_le[dev_e] += delta_all[dev_e]
    sig_le[dev_e] += delta_sig[dev_e]

    # edges 0 and 50 sit at the min/max: every element on the deciding side
    # is inside the repair band, so these counts are host-derived exactly
    # even if linspace's endpoints are off by an ulp.
    cnt_le[0] = TRU[:, 0].sum()
    sig_le[0] = TRU[is_sig, 0].sum()
    cnt_le[E - 1] = N - (len(ridx) - TRU[:, E - 1].sum())
    sig_le[E - 1] = ns_cnt - (int(is_sig.sum()) - TRU[is_sig, E - 1].sum())

    cnt_lt = cnt_le - t_all
    sig_lt = sig_le - t_sig

    ns_le = sig_le.astype(np.float32)
    ns_lt = sig_lt.astype(np.float32)
    nb_le = (cnt_le - sig_le).astype(np.float32)
    nb_lt = (cnt_lt - sig_lt).astype(np.float32)

    # ---- replicate the reference's tiny pair search (eager CPU jax) ---------
    with jax.default_device(cpu):
        ns_le_j = jnp.asarray(ns_le)
        ns_lt_j = jnp.asarray(ns_lt)
        nb_le_j = jnp.asarray(nb_le)
        nb_lt_j = jnp.asarray(nb_lt)
        n_f = jnp.float32(N)
        Ns = ns_le_j[-1]
        Nb = n_f - Ns

        hist0 = nb_le_j[1:] - nb_lt_j[:-1]
        hist1 = ns_le_j[1:] - ns_lt_j[:-1]

        gt0 = hist0 > hist1
        cand0 = jnp.logical_xor(gt0[:-1], gt0[1:]) & (hist0[:-1] > 0)
        gt1 = hist1 > hist0
        cand1 = jnp.logical_xor(gt1[:-1], gt1[1:]) & (hist1[:-1] > 0)
        mask = jnp.zeros((E,), bool).at[1:N_BINS].set(cand0 | cand1)
        cnt = jnp.sum(mask)
        mask = mask.at[-1].set(mask[-1] | (cnt == 1))

        a_c = -jnp.log1p(jnp.float32(-EPS))
        b_c = -jnp.log(jnp.float32(EPS))

        def bce(correct):
            return ((n_f - correct) * b_c + correct * a_c) / n_f

        c0 = ns_le_j + (Nb - nb_le_j)
        c1 = (Ns - ns_lt_j) + nb_lt_j
        c2 = (ns_le_j[None, :] - ns_lt_j[:, None]) + Nb - (
            nb_le_j[None, :] - nb_lt_j[:, None]
        )
        c3 = ns_le_j[:, None] + (Ns - ns_lt_j[None, :]) + (
            nb_le_j[None, :] - nb_lt_j[:, None]
        )

        L = jnp.stack(
            [
                jnp.broadcast_to(bce(c0)[:, None], (E, E)),
                jnp.broadcast_to(bce(c1)[:, None], (E, E)),
                bce(c2),
                bce(c3),
            ]
        )
        per_pair_min = jnp.min(L, axis=0)
        per_pair_case = jnp.argmin(L, axis=0)

        idxs = jnp.arange(E)
        valid = mask[:, None] & mask[None, :] & (idxs[:, None] < idxs[None, :])
        flat = jnp.argmin(jnp.where(valid, per_pair_min, jnp.inf))
        i = int(flat) // E
        j = int(flat) % E
        lower = np.float32(edges[i])
        upper = np.float32(edges[j])
        case = int(per_pair_case[i, j])

    # ---- L3: predicate ------------------------------------------------------
    prm = np.zeros((P, 8), np.float32)
    prm[:, 0] = lower
    prm[:, 1] = upper
    res3 = _run(
        f"pred{case}", [{"x": shards[c], "prm": prm} for c in CORE_IDS]
    )

    def true_pred(v):
        if case == 0:
            return v <= lower
        if case == 1:
            return v >= lower
        if case == 2:
            return (v >= lower) & (v <= upper)
        return (v <= lower) | (v >= upper)

    predc = np.empty(N, np.int32)
    predc[:N_DEV_TOT] = np.concatenate(
        [res3[c]["pred"] for c in CORE_IDS]
    ).astype(np.int32)
    predc[N_DEV_TOT:] = true_pred(tail).astype(np.int32)

    # patch every element whose fp16 compare vs lower/upper could disagree
    pband = 0.02 * h_step
    p_mask = (
        d_c
        | (np.abs(xt_true - np.float64(lower)) < pband)
        | (np.abs(xt_true - np.float64(upper)) < pband)
    )
    pidx = np.flatnonzero(p_mask)
    predc[pidx] = true_pred(xt_true[pidx]).astype(np.int32)

    out = np.empty(N, np.int32)
    out[perm] = predc
    return out
